# revision 53
# baseline (speedup 1.0000x reference)
"""Distributed Trainium2 kernel for the 4-block GNN (nn_ActorGNN).

Strategy (edge-parallel, dst-sharded), v3:
  - Pad N=100000 -> NP=100352 = 8 * 12544 nodes; core c owns nodes
    [c*12544, (c+1)*12544).  Node features live transposed in SBUF (H^T,
    bf16).
  - Algebra: the edge MLP  relu([x_src|x_dst|ea] @ We + be)  is split as
    relu(U[src] + V[dst] + ea@WeE + be) with U = x@WeS, V = x@WeD computed
    per node shard (cheap N-side matmuls).
  - U is AllGathered (fp8) in two segments (second half overlaps the tail
    windows of the previous block); every core gathers arbitrary source
    rows with SWDGE indirect DMA; V/agg stay core-local (edges live on the
    core that owns their destination).
  - Edges are grouped by destination window of 112 nodes and padded to a
    uniform 30 chunks x 128 edges per window (SPMD-uniform).  112 was
    chosen so the expand matmul's contraction packs [S^T(112) ; ea(16)]
    against [V_win(112) ; WeE(16)] - the edge-attr matmul rides along for
    free.  Per chunk PE does two matmuls: the merged expand (fp8 one-hot
    lhsT x bf16 vw) and the one-hot segment-reduce; the gathered U rows
    are folded in by the vector engine while it drains the expand PSUM to
    SBUF (bf16), and the scalar engine applies relu -> fp8.
  - The per-window dataflow is software-pipelined as a flat stream of
    chunk-groups with skew  exp(G) | add(G-1) | relu(G-2) | reduce(G-3)
    so the PE never waits for the DVE/ACT round-trip, and the per-window
    tail (scatter-mean, node update, next-block U/V) is staggered across
    the following window's group steps.
"""

import numpy as np
import ml_dtypes

BF16 = ml_dtypes.bfloat16
F8 = ml_dtypes.float8_e4m3

N = 100_000
E = 3_200_000
D = 128
ED = 16
NCORES = 8
NS = 12_544           # nodes per core
NP = NS * NCORES      # padded node count
WN = 112              # nodes per window (112 + 16 ea rows = 128 = PE K)
NWIN = NS // WN       # 112 windows per core
CHW = 30              # chunks (of 128 edges) per window, uniform
GRP = 6               # chunks per relu/expand-psum group
NG = CHW // GRP       # groups per window (5)
ES = NWIN * CHW * 128  # padded edge slots per core
WQ = 4                # windows per srcg slab load
NSEG = 2              # AllGather segments
NS2 = NS // NSEG      # AllGather segment size (nodes)
WSEG = NWIN // NSEG   # windows per AG segment
PF = 6                # windows of prefetch for edge slabs


def _seg_remap(g):
    """Map global node id -> row in the segment-ordered u_full layout."""
    r, j = g // NS, g % NS
    s, j2 = j // NS2, j % NS2
    return s * (NP // NSEG) + r * NS2 + j2


# ---------------------------------------------------------------------------
# host-side preparation
# ---------------------------------------------------------------------------

def _prep_edges(edge_index, edge_attr):
    """Distribute edges to cores/windows; build per-core slot arrays."""
    src = edge_index[0].astype(np.int64)
    dst = edge_index[1].astype(np.int64)

    cnt = np.bincount(dst, minlength=NP).astype(np.float32)
    invc_full = 1.0 / np.maximum(cnt, 1.0)

    core = dst // NS
    win = (dst % NS) // WN
    l = dst % WN

    per_core = []
    for c in range(NCORES):
        m = core == c
        s_c, w_c, l_c = src[m], win[m], l[m]
        order = np.argsort(w_c, kind="stable")
        s_c, w_c, l_c = s_c[order], w_c[order], l_c[order]
        ea_c = edge_attr[m][order]

        counts = np.bincount(w_c, minlength=NWIN)
        assert counts.max() <= CHW * 128, f"window overflow: {counts.max()}"
        starts = np.concatenate([[0], np.cumsum(counts)])

        # slot arrays, (chunk, partition) order inside each window
        srcg = np.zeros((128, NWIN * CHW), dtype=np.int32)
        lcol = np.full((128, NWIN * CHW), -1, dtype=np.int64)
        eat = np.zeros((ED, ES), dtype=np.float32)

        for w in range(NWIN):
            k = counts[w]
            sl = slice(starts[w], starts[w + 1])
            # sort window edges by src: gather descriptors walk HBM in
            # ascending address order (better DRAM page locality)
            ow = np.argsort(s_c[sl], kind="stable")
            sw, lw, ew_ = s_c[sl][ow], l_c[sl][ow], ea_c[sl][ow]
            j = np.arange(k)
            ch = w * CHW + j // 128
            p = j % 128
            srcg[p, ch] = sw
            lcol[p, ch] = lw
            eat[:, ch * 128 + p] = ew_.T

        # stk: stacked stationary [S^T(112) ; ea(16)] per edge slot, fp8
        stk = np.zeros((128, ES), dtype=F8)
        lflat = lcol.T.reshape(-1)  # slot s = g*128 + p
        pos_valid = np.nonzero(lflat >= 0)[0]
        stk[lflat[pos_valid], pos_valid] = 1.0
        stk[WN:] = eat.astype(F8)

        # ssl: reduce one-hot S per chunk [slot(128) x l(112)], fp8, packed
        # at 112-col pitch.  The reduce LDWEIGHTS reads a 128-col view that
        # overlaps 16 cols into the next chunk (junk accumulates into pw
        # rows 112:128, which are never read) so FWL still triggers.
        ssl = np.zeros((128, NWIN * CHW * 112 + 16), dtype=F8)
        pp = pos_valid % 128
        cc = pos_valid // 128
        ssl[pp, cc * 112 + lflat[pos_valid]] = 1.0

        # retile window-PAIR-major so each pair's slab is one contiguous
        # region in HBM with 2x-sized per-partition descriptors
        stk = np.ascontiguousarray(
            stk.reshape(128, NWIN // 2, 2 * CHW * 128).transpose(1, 0, 2))
        sslw = np.zeros((NWIN // 2, 128, 2 * CHW * 112 + 16), dtype=F8)
        for p in range(NWIN // 2):
            sslw[p, :, :2 * CHW * 112] = \
                ssl[:, p * 2 * CHW * 112:(p + 1) * 2 * CHW * 112]
        ssl = sslw

        per_core.append(
            dict(
                srcg=srcg.astype(np.int32),
                stk=stk,
                ssl=ssl,
                invc=invc_full[c * NS:(c + 1) * NS].reshape(NWIN, WN).T.copy(),
            )
        )
    return per_core


def _prep_inputs(inputs):
    x = inputs["x"]
    xp = np.zeros((NP, D), dtype=np.float32)
    xp[:N] = x
    per_core_edges = _prep_edges(np.asarray(inputs["edge_index"]),
                                 np.asarray(inputs["edge_attr"]))

    ones1 = np.ones((1, 128), dtype=np.float32)

    blocks = []
    for i in range(1, 5):
        We = np.asarray(inputs[f"We{i}"], np.float32)
        be = np.asarray(inputs[f"be{i}"], np.float32)
        Wn = np.asarray(inputs[f"Wn{i}"], np.float32)
        bn = np.asarray(inputs[f"bn{i}"], np.float32)
        din = We.shape[0] - ED
        din //= 2
        dout = We.shape[1]
        WeS, WeD, WeE = We[:din], We[din:2 * din], We[2 * din:]
        # pad dout -> 128
        wesd = np.zeros((128, 256), np.float32)
        wesd[:din, :dout] = WeS
        wesd[:din, 128:128 + dout] = WeD
        berow = np.zeros((1, 256), np.float32)
        berow[0, 128:128 + dout] = be
        wee = np.zeros((ED, 128), np.float32)
        wee[:, :dout] = WeE
        wnt = np.zeros((128, dout), np.float32)
        wnt[:din] = Wn[:din]
        wnb = np.zeros((128, dout), np.float32)
        wnb[:dout] = Wn[din:]
        bncol = bn.reshape(dout, 1).astype(np.float32)
        b = dict(wesd=wesd.astype(BF16), berow=berow.astype(BF16),
                 wee=wee.astype(BF16), wnt=wnt.astype(BF16),
                 wnb=wnb.astype(BF16), bn=bncol)
        if i == 4:
            # slim block-4 params: dout=1; pad the U/V pair to 64 columns so
            # the bounce rows are 128B (sub-burst HBM writes trigger RMW
            # stalls that throttled every DMA engine during block 3)
            wesd4b = np.zeros((128, 64), np.float32)
            wesd4b[:din, 0] = WeS[:, 0]
            wesd4b[:din, 1] = WeD[:, 0]
            berow4b = np.zeros((1, 64), np.float32)
            berow4b[0, 1] = be[0]
            b["wesdb"] = wesd4b.astype(BF16)
            b["berowb"] = berow4b.astype(BF16)
            b["weeb"] = np.tile(WeE[:, :1], (1, NWIN)).astype(BF16)
        blocks.append(b)

    in_maps = []
    for c in range(NCORES):
        m = dict(
            xT=xp[c * NS:(c + 1) * NS].T.astype(BF16).copy(),
            srcg=per_core_edges[c]["srcg"],
            stk=per_core_edges[c]["stk"],
            ssl=per_core_edges[c]["ssl"],
            invc=per_core_edges[c]["invc"],
            ones1=ones1.astype(BF16),
        )
        for i, b in enumerate(blocks, 1):
            for k, v in b.items():
                m[f"{k}{i}"] = v
        in_maps.append(m)
    return in_maps


# ---------------------------------------------------------------------------
# bass program
# ---------------------------------------------------------------------------

def _build():
    from concourse import bacc, bass, mybir, tile
    from concourse.masks import make_identity

    f32 = mybir.dt.float32
    bf16 = mybir.dt.bfloat16
    fp8 = mybir.dt.float8e4
    i32 = mybir.dt.int32

    nc = bacc.Bacc("TRN2", num_devices=NCORES)

    inp = {}
    for name, shape, dt in [
        ("xT", [128, NS], bf16),
        ("srcg", [128, NWIN * CHW], i32),
        ("stk", [NWIN // 2, 128, 2 * CHW * 128], fp8),
        ("ssl", [NWIN // 2, 128, 2 * CHW * 112 + 16], fp8),
        ("invc", [WN, NWIN], f32),
        ("ones1", [1, 128], bf16),
    ]:
        inp[name] = nc.dram_tensor(name, shape, dt, kind="ExternalInput")
    for i in range(1, 5):
        dout = 1 if i == 4 else D
        for name, shape, dt in [
            (f"wesd{i}", [128, 256], bf16),
            (f"berow{i}", [1, 256], bf16),
            (f"wee{i}", [ED, 128], bf16),
            (f"wnt{i}", [128, dout], bf16),
            (f"wnb{i}", [128, dout], bf16),
            (f"bn{i}", [dout, 1], f32),
        ]:
            inp[name] = nc.dram_tensor(name, shape, dt, kind="ExternalInput")
    for name, shape, dt in [
        ("wesdb4", [128, 64], bf16),
        ("berowb4", [1, 64], bf16),
        ("weeb4", [ED, NWIN], bf16),
    ]:
        inp[name] = nc.dram_tensor(name, shape, dt, kind="ExternalInput")
    out_ext = nc.dram_tensor("out", [1, NS], f32, kind="ExternalOutput")

    with tile.TileContext(nc) as tc:
        with tc.tile_pool(name="res", bufs=1) as res, \
             tc.tile_pool(name="sb", bufs=2) as sb, \
             tc.tile_pool(name="pexp", bufs=2, space="PSUM") as pexp, \
             tc.tile_pool(name="pnode", bufs=1, space="PSUM") as pnode, \
             tc.tile_pool(name="ppw", bufs=2, space="PSUM") as ppw, \
             tc.tile_pool(name="ppt", bufs=1, space="PSUM") as ppt, \
             tc.tile_pool(name="dram", bufs=1, space="DRAM") as dram:

            # resident tensors
            hT = res.tile([128, NS], bf16)         # node features, transposed
            # [V_win(112) ; WeE(16)] per window, window w at cols w*128
            vw = res.tile([128, NWIN * 128], bf16)
            vw4 = res.tile([128, NWIN], bf16)
            invc_sb = res.tile([WN, NWIN], f32)
            ones_sb = res.tile([1, 128], bf16)
            ident = res.tile([128, 128], bf16)

            nc.sync.dma_start(hT[:], inp["xT"][:])
            nc.sync.dma_start(invc_sb[:], inp["invc"][:])
            nc.sync.dma_start(ones_sb[:], inp["ones1"][:])
            nc.sync.dma_start(vw4[WN:128, :], inp["weeb4"][:])
            make_identity(nc, ident[:])

            # per-block weights, all resident
            wesd_sb, berow_sb, wee_sb, wnt_sb, wnb_sb, bn_sb = \
                {}, {}, {}, {}, {}, {}
            for i in range(1, 5):
                dout = 1 if i == 4 else D
                if i < 4:
                    wesd_sb[i] = res.tile([128, 256], bf16, tag=f"wesd{i}", name=f"wesd{i}")
                    berow_sb[i] = res.tile([1, 256], bf16, tag=f"berow{i}", name=f"berow{i}")
                    wee_sb[i] = res.tile([ED, 128], bf16, tag=f"wee{i}", name=f"wee{i}")
                    nc.sync.dma_start(wesd_sb[i][:], inp[f"wesd{i}"][:])
                    nc.sync.dma_start(berow_sb[i][:], inp[f"berow{i}"][:])
                    nc.sync.dma_start(wee_sb[i][:], inp[f"wee{i}"][:])
                else:
                    wesd_sb[i] = res.tile([128, 64], bf16, tag="wesd4", name="wesd4")
                    berow_sb[i] = res.tile([1, 64], bf16, tag="berow4", name="berow4")
                    nc.sync.dma_start(wesd_sb[i][:], inp["wesdb4"][:])
                    nc.sync.dma_start(berow_sb[i][:], inp["berowb4"][:])
                wnt_sb[i] = res.tile([128, dout], bf16, tag=f"wnt{i}", name=f"wnt{i}")
                wnb_sb[i] = res.tile([128, dout], bf16, tag=f"wnb{i}", name=f"wnb{i}")
                bn_sb[i] = res.tile([dout, 1], f32, tag=f"bn{i}", name=f"bn{i}")
                nc.sync.dma_start(wnt_sb[i][:], inp[f"wnt{i}"][:])
                nc.sync.dma_start(wnb_sb[i][:], inp[f"wnb{i}"][:])
                nc.sync.dma_start(bn_sb[i][:], inp[f"bn{i}"][:])

            u_bounce = dram.tile([NS, 128], fp8)
            u4_bounce = dram.tile([NS, 64], bf16, name="u4_bounce",
                                  tag="u4_bounce")

            u_fulls = {}
            for i in (3, 2, 1):  # reversed alloc order (block-3 DMA probe)
                uf = dram.tile([NP, 128], fp8, addr_space="Shared",
                               name=f"u_full{i}", tag=f"u_full{i}")
                u_fulls[i] = uf
            u4f = dram.tile([NP, 64], bf16, addr_space="Shared",
                            name="u4_full", tag="u4_full")
            u_fulls[4] = u4f

            def uv_phase(i, w, puv=None):
                """Compute U/V of block i for window w from current hT."""
                slim = i == 4
                wc = slice(w * WN, (w + 1) * WN)
                if puv is None:
                    puv = pnode.tile([128, 256], f32, tag="pnode",
                                     name="puv")
                nuv = 64 if slim else 256
                nc.tensor.matmul(out=puv[:WN, :nuv],
                                 lhsT=hT[:, wc], rhs=wesd_sb[i][:],
                                 start=True, stop=False)
                nc.tensor.matmul(out=puv[:WN, :nuv],
                                 lhsT=ones_sb[:, :WN],
                                 rhs=berow_sb[i][:], start=False, stop=True)
                if not slim:
                    utile = sb.tile([WN, 128], fp8, tag="utile")
                    nc.scalar.copy(utile[:], puv[:WN, :128])
                    nc.vector.tensor_copy(vw[:WN, w * 128:(w + 1) * 128],
                                          puv[:WN, 128:256])
                    nc.sync.dma_start(u_bounce[wc, :], utile[:])
                else:
                    utile = sb.tile([WN, 64], bf16, tag="utile4")
                    nc.scalar.copy(utile[:], puv[:WN, :64])
                    nc.vector.tensor_copy(vw4[:WN, w:w + 1], puv[:WN, 1:2])
                    nc.sync.dma_start(u4_bounce[wc, :], utile[:])

            def ag_all(i):
                """AllGather block i's U into u_fulls[i]."""
                full = u_fulls[i]
                in_ap = u4_bounce[:] if i == 4 else u_bounce[:]
                nc.gpsimd.collective_compute(
                    "AllGather", mybir.AluOpType.bypass,
                    replica_groups=[list(range(NCORES))],
                    ins=[in_ap.opt()],
                    outs=[full.opt()],
                )

            # ------------------------------------------------------------
            # pipelined block body
            # ------------------------------------------------------------
            state = {}

            def issue_loads(i, w):
                """Prefetch edge slabs for window w of block i."""
                slim = i == 4
                if w % WQ == 0:
                    srcg_sl = sb.tile([128, WQ * CHW], i32, tag="srcg",
                                      bufs=3)
                    nc.sync.dma_start(
                        srcg_sl[:],
                        inp["srcg"][:, w * CHW:(w + WQ) * CHW])
                    state["srcg"] = srcg_sl
                w0 = (w % WQ) * CHW
                uslab = sb.tile(
                    [128, CHW * 64] if slim else [128, CHW * 128],
                    bf16 if slim else fp8,
                    tag="uslab4" if slim else "uslab", bufs=PF + 2)
                nc.gpsimd.indirect_dma_start(
                    out=uslab[:],
                    out_offset=None,
                    in_=u_fulls[i][:],
                    in_offset=bass.IndirectOffsetOnAxis(
                        ap=state["srcg"][:, w0:w0 + CHW], axis=0),
                )
                if w % 2 == 0:
                    stslab = sb.tile([128, 2 * CHW * 128], fp8, tag="stslab",
                                     bufs=PF // 2 + 1)
                    nc.sync.dma_start(stslab[:], inp["stk"][w // 2])
                    sslab = sb.tile([128, 2 * CHW * 112 + 16], fp8,
                                    tag="sslab", bufs=PF // 2 + 2)
                    nc.sync.dma_start(sslab[:], inp["ssl"][w // 2])
                    state[("pair", w // 2)] = (stslab, sslab)
                state[("sl", w)] = (uslab,) + state[("pair", w // 2)]

            def exp_group(i, w, g):
                slim = i == 4
                _, stslab, _ = state[("sl", w)]
                g0 = g * GRP
                sb0 = (w % 2) * CHW * 128
                if not slim:
                    pe_ = pexp.tile([128, GRP * 128], f32, tag="pe")
                    for c in range(g0, g0 + GRP):
                        r = (c - g0) * 128
                        nc.tensor.matmul(
                            out=pe_[:, r:r + 128],
                            lhsT=stslab[:, sb0 + c * 128:sb0 + (c + 1) * 128],
                            rhs=vw[:, w * 128:(w + 1) * 128],
                            start=True, stop=True)
                else:
                    pe_ = pexp.tile([128, GRP], f32, tag="pe")
                    for c in range(g0, g0 + GRP):
                        nc.tensor.matmul(
                            out=pe_[:, c - g0:c - g0 + 1],
                            lhsT=stslab[:, sb0 + c * 128:sb0 + (c + 1) * 128],
                            rhs=vw4[:, w:w + 1],
                            start=True, stop=True)
                state[("pe", w, g)] = pe_

            def add_group(i, w, g):
                """Drain expand PSUM + add gathered U rows -> SBUF bf16."""
                slim = i == 4
                uslab, _, _ = state[("sl", w)]
                pe_ = state.pop(("pe", w, g))
                g0 = g * GRP
                if not slim:
                    smt = sb.tile([128, GRP * 128], bf16, tag="smt", bufs=4)
                    nc.vector.tensor_tensor(
                        out=smt[:], in0=pe_[:],
                        in1=uslab[:, g0 * 128:(g0 + GRP) * 128],
                        op=mybir.AluOpType.add)
                else:
                    smt = sb.tile([128, GRP], bf16, tag="smt4", bufs=4)
                    u4r = uslab[:].rearrange("p (c t) -> p c t", t=64)
                    nc.vector.tensor_tensor(
                        out=smt[:], in0=pe_[:],
                        in1=u4r[:, g0:g0 + GRP, 0:1],
                        op=mybir.AluOpType.add)
                state[("sm", w, g)] = smt

            def relu_group(i, w, g):
                slim = i == 4
                smt = state.pop(("sm", w, g))
                if not slim:
                    wslab = sb.tile([128, GRP * 128], bf16, tag="wslab",
                                    bufs=4)
                else:
                    wslab = sb.tile([128, GRP], bf16, tag="wslab4", bufs=4)
                if g == NG - 1 and not slim:
                    # one group per window on DVE (4x mode, 16-bit SBUF)
                    nc.vector.tensor_scalar(
                        out=wslab[:], in0=smt[:], scalar1=0.0, scalar2=None,
                        op0=mybir.AluOpType.max)
                else:
                    nc.scalar.activation(
                        wslab[:], smt[:], mybir.ActivationFunctionType.Relu)
                state[("ws", w, g)] = wslab

            def red_group(i, w, g):
                slim = i == 4
                _, _, sslab = state[("sl", w)]
                wslab = state.pop(("ws", w, g))
                if g == 0:
                    state[("pw", w)] = ppw.tile([128, 128], f32, tag="pw",
                                                name="pw")
                pw = state[("pw", w)]
                g0 = g * GRP
                rb0 = (w % 2) * CHW * 112
                for c in range(g0, g0 + GRP):
                    cc = c - g0
                    if not slim:
                        nc.tensor.matmul(
                            out=pw[:, :],
                            lhsT=sslab[:, rb0 + c * 112:rb0 + c * 112 + 128],
                            rhs=wslab[:, cc * 128:(cc + 1) * 128],
                            start=(c == 0), stop=(c == CHW - 1))
                    else:
                        nc.tensor.matmul(
                            out=pw[:, :1],
                            lhsT=sslab[:, rb0 + c * 112:rb0 + c * 112 + 128],
                            rhs=wslab[:, cc:cc + 1],
                            start=(c == 0), stop=(c == CHW - 1))
                if g == NG - 1:
                    state.pop(("sl", w))
                    if w % 2 == 1:
                        state.pop(("pair", w // 2))

            def tail1(i, w):
                """scatter-mean scale + transpose."""
                slim = i == 4
                nd = 1 if slim else 128
                pw = state.pop(("pw", w))
                argm = sb.tile([WN, nd], bf16,
                               tag="argm4" if slim else "argm", bufs=2)
                # drain + scatter-mean scale in one ACT op (per-partition
                # scale port carries 1/max(cnt,1))
                nc.scalar.activation(
                    argm[:], pw[:WN, :nd],
                    mybir.ActivationFunctionType.Identity,
                    scale=invc_sb[:, w:w + 1])
                pt = ppt.tile([nd, WN], bf16, tag="pt")
                nc.tensor.transpose(pt[:], argm[:], ident[:WN, :WN])
                state[("pt", w)] = pt

            def tail2(i, w):
                """aggregate -> node update -> new hT (or sigmoid out)."""
                slim = i == 4
                dout = 1 if slim else D
                nd = 1 if slim else 128
                pt = state.pop(("pt", w))
                aggt = sb.tile([128, WN], bf16, tag="aggt", bufs=2)
                nc.scalar.copy(aggt[:nd, :], pt[:])
                wc = slice(w * WN, (w + 1) * WN)
                pupd = pnode.tile([128, 128], f32, tag="pnode")
                nc.tensor.matmul(out=pupd[:dout, :WN], lhsT=wnt_sb[i][:],
                                 rhs=hT[:, wc], start=True, stop=False)
                nc.tensor.matmul(out=pupd[:dout, :WN], lhsT=wnb_sb[i][:],
                                 rhs=aggt[:], start=False, stop=True)
                if not slim:
                    nc.scalar.activation(
                        hT[:, wc], pupd[:, :WN],
                        mybir.ActivationFunctionType.Relu,
                        bias=bn_sb[i][:])
                else:
                    out_t = sb.tile([1, WN], f32, tag="out_t")
                    nc.scalar.activation(
                        out_t[:], pupd[:1, :WN],
                        mybir.ActivationFunctionType.Sigmoid,
                        bias=bn_sb[i][:])
                    nc.sync.dma_start(out_ext[:, wc], out_t[:])

            def tail3(i, w):
                """U/V of block i+1 for window w + segment AllGathers."""
                if i < 4:
                    uv_phase(i + 1, w)
                    if w == NWIN - 1:
                        ag_all(i + 1)

            def wee_bcast(i):
                """WeE of block i into rows 112:128 of every vw col block."""
                nc.sync.dma_start(
                    vw[WN:128, :].rearrange("p (w d) -> p w d", w=NWIN),
                    inp[f"wee{i}"][:, None, :].to_broadcast([ED, NWIN, 128]))

            # ---- prologue: UV of block 1 from x ----
            # five UV windows in flight: pnode + 2 pexp buffers x 2 slots
            wee_bcast(1)
            pe_hold = None
            for w in range(NWIN):
                m = w % 5
                if m == 0:
                    uv_phase(1, w)
                else:
                    if m in (1, 3):
                        pe_hold = pexp.tile([128, GRP * 128], f32, tag="pe",
                                            name="puv_alt")
                        uv_phase(1, w, puv=pe_hold[:, 0:256])
                    else:
                        uv_phase(1, w, puv=pe_hold[:, 256:512])
            ag_all(1)

            # ---- pipelined blocks ----
            for i in range(1, 5):
                for w in range(PF):
                    issue_loads(i, w)
                # flat stream of group-steps with skew:
                #   exp(G) | add(G-1) | relu(G-2) | red(G-3)
                # tails for window w ride at steps (w+1, 3), (w+1, 4),
                # (w+2, 0) of the stream.
                TOT = (NWIN + 2) * NG  # flush room
                for G in range(TOT):
                    w, g = divmod(G, NG)
                    if g == 0 and w + PF < NWIN:
                        issue_loads(i, w + PF)
                    if w < NWIN:
                        exp_group(i, w, g)
                    for (dk, fn) in ((1, add_group), (2, relu_group),
                                     (3, red_group)):
                        Gp = G - dk
                        if Gp >= 0:
                            wp, gp = divmod(Gp, NG)
                            if wp < NWIN:
                                fn(i, wp, gp)
                    # staggered tails: tail1(w-1)@g3, tail2(w-1)@g4,
                    # tail3(w-2)@g0
                    if g == 3 and 0 <= w - 1 < NWIN:
                        tail1(i, w - 1)
                    if g == 4 and 0 <= w - 1 < NWIN:
                        tail2(i, w - 1)
                    if g == 0 and 0 <= w - 2 < NWIN:
                        tail3(i, w - 2)
                if i < 3:
                    wee_bcast(i + 1)

    nc.finalize()
    return nc


_NC_CACHE = {}


def kernel(**inputs):
    from concourse.bass_utils import run_bass_kernel_spmd

    in_maps = _prep_inputs(inputs)
    if "nc" not in _NC_CACHE:
        _NC_CACHE["nc"] = _build()
    nc = _NC_CACHE["nc"]
    res = run_bass_kernel_spmd(nc, in_maps, core_ids=list(range(NCORES)))
    outs = [res.results[c]["out"].reshape(-1) for c in range(NCORES)]
    return np.concatenate(outs)[:N].reshape(N, 1).astype(np.float32)


# revision 54
# speedup vs baseline: 1.0165x; 1.0165x over previous
"""Distributed Trainium2 kernel for the 4-block GNN (nn_ActorGNN).

Strategy (edge-parallel, dst-sharded), v3:
  - Pad N=100000 -> NP=100352 = 8 * 12544 nodes; core c owns nodes
    [c*12544, (c+1)*12544).  Node features live transposed in SBUF (H^T,
    bf16).
  - Algebra: the edge MLP  relu([x_src|x_dst|ea] @ We + be)  is split as
    relu(U[src] + V[dst] + ea@WeE + be) with U = x@WeS, V = x@WeD computed
    per node shard (cheap N-side matmuls).
  - U is AllGathered (fp8) in two segments (second half overlaps the tail
    windows of the previous block); every core gathers arbitrary source
    rows with SWDGE indirect DMA; V/agg stay core-local (edges live on the
    core that owns their destination).
  - Edges are grouped by destination window of 112 nodes and padded to a
    uniform 30 chunks x 128 edges per window (SPMD-uniform).  112 was
    chosen so the expand matmul's contraction packs [S^T(112) ; ea(16)]
    against [V_win(112) ; WeE(16)] - the edge-attr matmul rides along for
    free.  Per chunk PE does two matmuls: the merged expand (fp8 one-hot
    lhsT x bf16 vw) and the one-hot segment-reduce; the gathered U rows
    are folded in by the vector engine while it drains the expand PSUM to
    SBUF (bf16), and the scalar engine applies relu -> fp8.
  - The per-window dataflow is software-pipelined as a flat stream of
    chunk-groups with skew  exp(G) | add(G-1) | relu(G-2) | reduce(G-3)
    so the PE never waits for the DVE/ACT round-trip, and the per-window
    tail (scatter-mean, node update, next-block U/V) is staggered across
    the following window's group steps.
"""

import numpy as np
import ml_dtypes

BF16 = ml_dtypes.bfloat16
F8 = ml_dtypes.float8_e4m3

N = 100_000
E = 3_200_000
D = 128
ED = 16
NCORES = 8
NS = 12_544           # nodes per core
NP = NS * NCORES      # padded node count
WN = 112              # nodes per window (112 + 16 ea rows = 128 = PE K)
NWIN = NS // WN       # 112 windows per core
CHW = 30              # chunks (of 128 edges) per window, uniform
GRP = 6               # chunks per relu/expand-psum group
NG = CHW // GRP       # groups per window (5)
ES = NWIN * CHW * 128  # padded edge slots per core
WQ = 4                # windows per srcg slab load
NSEG = 2              # AllGather segments
NS2 = NS // NSEG      # AllGather segment size (nodes)
WSEG = NWIN // NSEG   # windows per AG segment
PF = 4                # windows of prefetch for edge slabs


def _seg_remap(g):
    """Map global node id -> row in the segment-ordered u_full layout."""
    r, j = g // NS, g % NS
    s, j2 = j // NS2, j % NS2
    return s * (NP // NSEG) + r * NS2 + j2


# ---------------------------------------------------------------------------
# host-side preparation
# ---------------------------------------------------------------------------

def _prep_edges(edge_index, edge_attr):
    """Distribute edges to cores/windows; build per-core slot arrays."""
    src = edge_index[0].astype(np.int64)
    dst = edge_index[1].astype(np.int64)

    cnt = np.bincount(dst, minlength=NP).astype(np.float32)
    invc_full = 1.0 / np.maximum(cnt, 1.0)

    core = dst // NS
    win = (dst % NS) // WN
    l = dst % WN

    per_core = []
    for c in range(NCORES):
        m = core == c
        s_c, w_c, l_c = src[m], win[m], l[m]
        order = np.argsort(w_c, kind="stable")
        s_c, w_c, l_c = s_c[order], w_c[order], l_c[order]
        ea_c = edge_attr[m][order]

        counts = np.bincount(w_c, minlength=NWIN)
        assert counts.max() <= CHW * 128, f"window overflow: {counts.max()}"
        starts = np.concatenate([[0], np.cumsum(counts)])

        # slot arrays, (chunk, partition) order inside each window
        srcg = np.zeros((128, NWIN * CHW), dtype=np.int32)
        lcol = np.full((128, NWIN * CHW), -1, dtype=np.int64)
        eat = np.zeros((ED, ES), dtype=np.float32)

        for w in range(NWIN):
            k = counts[w]
            sl = slice(starts[w], starts[w + 1])
            # sort window edges by src: gather descriptors walk HBM in
            # ascending address order (better DRAM page locality)
            ow = np.argsort(s_c[sl], kind="stable")
            sw, lw, ew_ = s_c[sl][ow], l_c[sl][ow], ea_c[sl][ow]
            j = np.arange(k)
            ch = w * CHW + j // 128
            p = j % 128
            srcg[p, ch] = sw
            lcol[p, ch] = lw
            eat[:, ch * 128 + p] = ew_.T

        # stk: stacked stationary [S^T(112) ; ea(16)] per edge slot, fp8
        stk = np.zeros((128, ES), dtype=F8)
        lflat = lcol.T.reshape(-1)  # slot s = g*128 + p
        pos_valid = np.nonzero(lflat >= 0)[0]
        stk[lflat[pos_valid], pos_valid] = 1.0
        stk[WN:] = eat.astype(F8)

        # ssl: reduce one-hot S per chunk [slot(128) x l(112)], fp8, packed
        # at 112-col pitch.  The reduce LDWEIGHTS reads a 128-col view that
        # overlaps 16 cols into the next chunk (junk accumulates into pw
        # rows 112:128, which are never read) so FWL still triggers.
        ssl = np.zeros((128, NWIN * CHW * 112 + 16), dtype=F8)
        pp = pos_valid % 128
        cc = pos_valid // 128
        ssl[pp, cc * 112 + lflat[pos_valid]] = 1.0

        # retile window-PAIR-major so each pair's slab is one contiguous
        # region in HBM with 2x-sized per-partition descriptors
        stk = np.ascontiguousarray(
            stk.reshape(128, NWIN // 2, 2 * CHW * 128).transpose(1, 0, 2))
        sslw = np.zeros((NWIN // 2, 128, 2 * CHW * 112 + 16), dtype=F8)
        for p in range(NWIN // 2):
            sslw[p, :, :2 * CHW * 112] = \
                ssl[:, p * 2 * CHW * 112:(p + 1) * 2 * CHW * 112]
        ssl = sslw

        per_core.append(
            dict(
                srcg=srcg.astype(np.int32),
                stk=stk,
                ssl=ssl,
                invc=invc_full[c * NS:(c + 1) * NS].reshape(NWIN, WN).T.copy(),
            )
        )
    return per_core


def _prep_inputs(inputs):
    x = inputs["x"]
    xp = np.zeros((NP, D), dtype=np.float32)
    xp[:N] = x
    per_core_edges = _prep_edges(np.asarray(inputs["edge_index"]),
                                 np.asarray(inputs["edge_attr"]))

    ones1 = np.ones((1, 128), dtype=np.float32)

    blocks = []
    for i in range(1, 5):
        We = np.asarray(inputs[f"We{i}"], np.float32)
        be = np.asarray(inputs[f"be{i}"], np.float32)
        Wn = np.asarray(inputs[f"Wn{i}"], np.float32)
        bn = np.asarray(inputs[f"bn{i}"], np.float32)
        din = We.shape[0] - ED
        din //= 2
        dout = We.shape[1]
        WeS, WeD, WeE = We[:din], We[din:2 * din], We[2 * din:]
        # pad dout -> 128
        wesd = np.zeros((128, 256), np.float32)
        wesd[:din, :dout] = WeS
        wesd[:din, 128:128 + dout] = WeD
        berow = np.zeros((1, 256), np.float32)
        berow[0, 128:128 + dout] = be
        wee = np.zeros((ED, 128), np.float32)
        wee[:, :dout] = WeE
        wnt = np.zeros((128, dout), np.float32)
        wnt[:din] = Wn[:din]
        wnb = np.zeros((128, dout), np.float32)
        wnb[:dout] = Wn[din:]
        bncol = bn.reshape(dout, 1).astype(np.float32)
        b = dict(wesd=wesd.astype(BF16), berow=berow.astype(BF16),
                 wee=wee.astype(BF16), wnt=wnt.astype(BF16),
                 wnb=wnb.astype(BF16), bn=bncol)
        if i == 4:
            # slim block-4 params: dout=1; pad the U/V pair to 64 columns so
            # the bounce rows are 128B (sub-burst HBM writes trigger RMW
            # stalls that throttled every DMA engine during block 3)
            wesd4b = np.zeros((128, 64), np.float32)
            wesd4b[:din, 0] = WeS[:, 0]
            wesd4b[:din, 1] = WeD[:, 0]
            berow4b = np.zeros((1, 64), np.float32)
            berow4b[0, 1] = be[0]
            b["wesdb"] = wesd4b.astype(BF16)
            b["berowb"] = berow4b.astype(BF16)
            b["weeb"] = np.tile(WeE[:, :1], (1, NWIN)).astype(BF16)
        blocks.append(b)

    in_maps = []
    for c in range(NCORES):
        m = dict(
            xT=xp[c * NS:(c + 1) * NS].T.astype(BF16).copy(),
            srcg=per_core_edges[c]["srcg"],
            stk=per_core_edges[c]["stk"],
            ssl=per_core_edges[c]["ssl"],
            invc=per_core_edges[c]["invc"],
            ones1=ones1.astype(BF16),
        )
        for i, b in enumerate(blocks, 1):
            for k, v in b.items():
                m[f"{k}{i}"] = v
        in_maps.append(m)
    return in_maps


# ---------------------------------------------------------------------------
# bass program
# ---------------------------------------------------------------------------

def _build():
    from concourse import bacc, bass, mybir, tile
    from concourse.masks import make_identity

    f32 = mybir.dt.float32
    bf16 = mybir.dt.bfloat16
    fp8 = mybir.dt.float8e4
    i32 = mybir.dt.int32

    nc = bacc.Bacc("TRN2", num_devices=NCORES)

    inp = {}
    for name, shape, dt in [
        ("xT", [128, NS], bf16),
        ("srcg", [128, NWIN * CHW], i32),
        ("stk", [NWIN // 2, 128, 2 * CHW * 128], fp8),
        ("ssl", [NWIN // 2, 128, 2 * CHW * 112 + 16], fp8),
        ("invc", [WN, NWIN], f32),
        ("ones1", [1, 128], bf16),
    ]:
        inp[name] = nc.dram_tensor(name, shape, dt, kind="ExternalInput")
    for i in range(1, 5):
        dout = 1 if i == 4 else D
        for name, shape, dt in [
            (f"wesd{i}", [128, 256], bf16),
            (f"berow{i}", [1, 256], bf16),
            (f"wee{i}", [ED, 128], bf16),
            (f"wnt{i}", [128, dout], bf16),
            (f"wnb{i}", [128, dout], bf16),
            (f"bn{i}", [dout, 1], f32),
        ]:
            inp[name] = nc.dram_tensor(name, shape, dt, kind="ExternalInput")
    for name, shape, dt in [
        ("wesdb4", [128, 64], bf16),
        ("berowb4", [1, 64], bf16),
        ("weeb4", [ED, NWIN], bf16),
    ]:
        inp[name] = nc.dram_tensor(name, shape, dt, kind="ExternalInput")
    out_ext = nc.dram_tensor("out", [1, NS], f32, kind="ExternalOutput")

    with tile.TileContext(nc) as tc:
        with tc.tile_pool(name="res", bufs=1) as res, \
             tc.tile_pool(name="sb", bufs=2) as sb, \
             tc.tile_pool(name="pexp", bufs=2, space="PSUM") as pexp, \
             tc.tile_pool(name="pnode", bufs=1, space="PSUM") as pnode, \
             tc.tile_pool(name="ppw", bufs=2, space="PSUM") as ppw, \
             tc.tile_pool(name="ppt", bufs=1, space="PSUM") as ppt, \
             tc.tile_pool(name="dram", bufs=1, space="DRAM") as dram:

            # resident tensors
            hT = res.tile([128, NS], bf16)         # node features, transposed
            # [V_win(112) ; WeE(16)] per window, window w at cols w*128
            vw = res.tile([128, NWIN * 128], bf16)
            vw4 = res.tile([128, NWIN], bf16)
            invc_sb = res.tile([WN, NWIN], f32)
            ones_sb = res.tile([1, 128], bf16)
            ident = res.tile([128, 128], bf16)

            nc.sync.dma_start(hT[:], inp["xT"][:])
            nc.sync.dma_start(invc_sb[:], inp["invc"][:])
            nc.sync.dma_start(ones_sb[:], inp["ones1"][:])
            nc.sync.dma_start(vw4[WN:128, :], inp["weeb4"][:])
            make_identity(nc, ident[:])

            # per-block weights, all resident
            wesd_sb, berow_sb, wee_sb, wnt_sb, wnb_sb, bn_sb = \
                {}, {}, {}, {}, {}, {}
            for i in range(1, 5):
                dout = 1 if i == 4 else D
                if i < 4:
                    wesd_sb[i] = res.tile([128, 256], bf16, tag=f"wesd{i}", name=f"wesd{i}")
                    berow_sb[i] = res.tile([1, 256], bf16, tag=f"berow{i}", name=f"berow{i}")
                    wee_sb[i] = res.tile([ED, 128], bf16, tag=f"wee{i}", name=f"wee{i}")
                    nc.sync.dma_start(wesd_sb[i][:], inp[f"wesd{i}"][:])
                    nc.sync.dma_start(berow_sb[i][:], inp[f"berow{i}"][:])
                    nc.sync.dma_start(wee_sb[i][:], inp[f"wee{i}"][:])
                else:
                    wesd_sb[i] = res.tile([128, 64], bf16, tag="wesd4", name="wesd4")
                    berow_sb[i] = res.tile([1, 64], bf16, tag="berow4", name="berow4")
                    nc.sync.dma_start(wesd_sb[i][:], inp["wesdb4"][:])
                    nc.sync.dma_start(berow_sb[i][:], inp["berowb4"][:])
                wnt_sb[i] = res.tile([128, dout], bf16, tag=f"wnt{i}", name=f"wnt{i}")
                wnb_sb[i] = res.tile([128, dout], bf16, tag=f"wnb{i}", name=f"wnb{i}")
                bn_sb[i] = res.tile([dout, 1], f32, tag=f"bn{i}", name=f"bn{i}")
                nc.sync.dma_start(wnt_sb[i][:], inp[f"wnt{i}"][:])
                nc.sync.dma_start(wnb_sb[i][:], inp[f"wnb{i}"][:])
                nc.sync.dma_start(bn_sb[i][:], inp[f"bn{i}"][:])

            u_bounce = dram.tile([NS, 128], fp8)
            u4_bounce = dram.tile([NS, 64], bf16, name="u4_bounce",
                                  tag="u4_bounce")
            u4_pack = dram.tile([NS, 2], bf16, name="u4_pack",
                                tag="u4_pack")

            u_fulls = {}
            for i in (3, 2, 1):  # reversed alloc order (block-3 DMA probe)
                uf = dram.tile([NP, 128], fp8, addr_space="Shared",
                               name=f"u_full{i}", tag=f"u_full{i}")
                u_fulls[i] = uf
            u4f = dram.tile([NP, 2], bf16, addr_space="Shared",
                            name="u4_full", tag="u4_full")
            u_fulls[4] = u4f

            def uv_phase(i, w, puv=None):
                """Compute U/V of block i for window w from current hT."""
                slim = i == 4
                wc = slice(w * WN, (w + 1) * WN)
                if puv is None:
                    puv = pnode.tile([128, 256], f32, tag="pnode",
                                     name="puv")
                nuv = 64 if slim else 256
                nc.tensor.matmul(out=puv[:WN, :nuv],
                                 lhsT=hT[:, wc], rhs=wesd_sb[i][:],
                                 start=True, stop=False)
                nc.tensor.matmul(out=puv[:WN, :nuv],
                                 lhsT=ones_sb[:, :WN],
                                 rhs=berow_sb[i][:], start=False, stop=True)
                if not slim:
                    utile = sb.tile([WN, 128], fp8, tag="utile")
                    nc.scalar.copy(utile[:], puv[:WN, :128])
                    nc.vector.tensor_copy(vw[:WN, w * 128:(w + 1) * 128],
                                          puv[:WN, 128:256])
                    nc.sync.dma_start(u_bounce[wc, :], utile[:])
                else:
                    utile = sb.tile([WN, 64], bf16, tag="utile4")
                    nc.scalar.copy(utile[:], puv[:WN, :64])
                    nc.vector.tensor_copy(vw4[:WN, w:w + 1], puv[:WN, 1:2])
                    nc.sync.dma_start(u4_bounce[wc, :], utile[:])

            def ag_all(i):
                """AllGather block i's U into u_fulls[i]."""
                full = u_fulls[i]
                if i == 4:
                    # repack the 128B-row bounce (RMW-free writes) into the
                    # compact layout the AllGather/gather expects
                    nc.sync.dma_start(u4_pack[:], u4_bounce[:, 0:2])
                    in_ap = u4_pack[:]
                else:
                    in_ap = u_bounce[:]
                nc.gpsimd.collective_compute(
                    "AllGather", mybir.AluOpType.bypass,
                    replica_groups=[list(range(NCORES))],
                    ins=[in_ap.opt()],
                    outs=[full.opt()],
                )

            # ------------------------------------------------------------
            # pipelined block body
            # ------------------------------------------------------------
            state = {}

            def issue_loads(i, w):
                """Prefetch edge slabs for window w of block i."""
                slim = i == 4
                if w % WQ == 0:
                    srcg_sl = sb.tile([128, WQ * CHW], i32, tag="srcg",
                                      bufs=3)
                    nc.sync.dma_start(
                        srcg_sl[:],
                        inp["srcg"][:, w * CHW:(w + WQ) * CHW])
                    state["srcg"] = srcg_sl
                w0 = (w % WQ) * CHW
                uslab = sb.tile(
                    [128, CHW * 2] if slim else [128, CHW * 128],
                    bf16 if slim else fp8,
                    tag="uslab4" if slim else "uslab", bufs=PF + 2)
                nc.gpsimd.indirect_dma_start(
                    out=uslab[:],
                    out_offset=None,
                    in_=u_fulls[i][:],
                    in_offset=bass.IndirectOffsetOnAxis(
                        ap=state["srcg"][:, w0:w0 + CHW], axis=0),
                )
                if w % 2 == 0:
                    stslab = sb.tile([128, 2 * CHW * 128], fp8, tag="stslab",
                                     bufs=PF // 2 + 1)
                    nc.sync.dma_start(stslab[:], inp["stk"][w // 2])
                    sslab = sb.tile([128, 2 * CHW * 112 + 16], fp8,
                                    tag="sslab", bufs=PF // 2 + 2)
                    nc.sync.dma_start(sslab[:], inp["ssl"][w // 2])
                    state[("pair", w // 2)] = (stslab, sslab)
                state[("sl", w)] = (uslab,) + state[("pair", w // 2)]

            def exp_group(i, w, g):
                slim = i == 4
                _, stslab, _ = state[("sl", w)]
                g0 = g * GRP
                sb0 = (w % 2) * CHW * 128
                if not slim:
                    pe_ = pexp.tile([128, GRP * 128], f32, tag="pe")
                    for c in range(g0, g0 + GRP):
                        r = (c - g0) * 128
                        nc.tensor.matmul(
                            out=pe_[:, r:r + 128],
                            lhsT=stslab[:, sb0 + c * 128:sb0 + (c + 1) * 128],
                            rhs=vw[:, w * 128:(w + 1) * 128],
                            start=True, stop=True)
                else:
                    pe_ = pexp.tile([128, GRP], f32, tag="pe")
                    for c in range(g0, g0 + GRP):
                        nc.tensor.matmul(
                            out=pe_[:, c - g0:c - g0 + 1],
                            lhsT=stslab[:, sb0 + c * 128:sb0 + (c + 1) * 128],
                            rhs=vw4[:, w:w + 1],
                            start=True, stop=True)
                state[("pe", w, g)] = pe_

            def add_group(i, w, g):
                """Drain expand PSUM + add gathered U rows -> SBUF bf16."""
                slim = i == 4
                uslab, _, _ = state[("sl", w)]
                pe_ = state.pop(("pe", w, g))
                g0 = g * GRP
                if not slim:
                    smt = sb.tile([128, GRP * 128], bf16, tag="smt", bufs=4)
                    nc.vector.tensor_tensor(
                        out=smt[:], in0=pe_[:],
                        in1=uslab[:, g0 * 128:(g0 + GRP) * 128],
                        op=mybir.AluOpType.add)
                else:
                    smt = sb.tile([128, GRP], bf16, tag="smt4", bufs=4)
                    u4r = uslab[:].rearrange("p (c t) -> p c t", t=2)
                    nc.vector.tensor_tensor(
                        out=smt[:], in0=pe_[:],
                        in1=u4r[:, g0:g0 + GRP, 0:1],
                        op=mybir.AluOpType.add)
                state[("sm", w, g)] = smt

            def relu_group(i, w, g):
                slim = i == 4
                smt = state.pop(("sm", w, g))
                if not slim:
                    wslab = sb.tile([128, GRP * 128], bf16, tag="wslab",
                                    bufs=4)
                else:
                    wslab = sb.tile([128, GRP], bf16, tag="wslab4", bufs=4)
                if g == NG - 1 and not slim:
                    # one group per window on DVE (4x mode, 16-bit SBUF)
                    nc.vector.tensor_scalar(
                        out=wslab[:], in0=smt[:], scalar1=0.0, scalar2=None,
                        op0=mybir.AluOpType.max)
                else:
                    nc.scalar.activation(
                        wslab[:], smt[:], mybir.ActivationFunctionType.Relu)
                state[("ws", w, g)] = wslab

            def red_group(i, w, g):
                slim = i == 4
                _, _, sslab = state[("sl", w)]
                wslab = state.pop(("ws", w, g))
                if g == 0:
                    state[("pw", w)] = ppw.tile([128, 128], f32, tag="pw",
                                                name="pw")
                pw = state[("pw", w)]
                g0 = g * GRP
                rb0 = (w % 2) * CHW * 112
                for c in range(g0, g0 + GRP):
                    cc = c - g0
                    if not slim:
                        nc.tensor.matmul(
                            out=pw[:, :],
                            lhsT=sslab[:, rb0 + c * 112:rb0 + c * 112 + 128],
                            rhs=wslab[:, cc * 128:(cc + 1) * 128],
                            start=(c == 0), stop=(c == CHW - 1))
                    else:
                        nc.tensor.matmul(
                            out=pw[:, :1],
                            lhsT=sslab[:, rb0 + c * 112:rb0 + c * 112 + 128],
                            rhs=wslab[:, cc:cc + 1],
                            start=(c == 0), stop=(c == CHW - 1))
                if g == NG - 1:
                    state.pop(("sl", w))
                    if w % 2 == 1:
                        state.pop(("pair", w // 2))

            def tail1(i, w):
                """scatter-mean scale + transpose."""
                slim = i == 4
                nd = 1 if slim else 128
                pw = state.pop(("pw", w))
                argm = sb.tile([WN, nd], bf16,
                               tag="argm4" if slim else "argm", bufs=2)
                # drain + scatter-mean scale in one ACT op (per-partition
                # scale port carries 1/max(cnt,1))
                nc.scalar.activation(
                    argm[:], pw[:WN, :nd],
                    mybir.ActivationFunctionType.Identity,
                    scale=invc_sb[:, w:w + 1])
                pt = ppt.tile([nd, WN], bf16, tag="pt")
                nc.tensor.transpose(pt[:], argm[:], ident[:WN, :WN])
                state[("pt", w)] = pt

            def tail2(i, w):
                """aggregate -> node update -> new hT (or sigmoid out)."""
                slim = i == 4
                dout = 1 if slim else D
                nd = 1 if slim else 128
                pt = state.pop(("pt", w))
                aggt = sb.tile([128, WN], bf16, tag="aggt", bufs=2)
                nc.scalar.copy(aggt[:nd, :], pt[:])
                wc = slice(w * WN, (w + 1) * WN)
                pupd = pnode.tile([128, 128], f32, tag="pnode")
                nc.tensor.matmul(out=pupd[:dout, :WN], lhsT=wnt_sb[i][:],
                                 rhs=hT[:, wc], start=True, stop=False)
                nc.tensor.matmul(out=pupd[:dout, :WN], lhsT=wnb_sb[i][:],
                                 rhs=aggt[:], start=False, stop=True)
                if not slim:
                    nc.scalar.activation(
                        hT[:, wc], pupd[:, :WN],
                        mybir.ActivationFunctionType.Relu,
                        bias=bn_sb[i][:])
                else:
                    out_t = sb.tile([1, WN], f32, tag="out_t")
                    nc.scalar.activation(
                        out_t[:], pupd[:1, :WN],
                        mybir.ActivationFunctionType.Sigmoid,
                        bias=bn_sb[i][:])
                    nc.sync.dma_start(out_ext[:, wc], out_t[:])

            def tail3(i, w):
                """U/V of block i+1 for window w + segment AllGathers."""
                if i < 4:
                    uv_phase(i + 1, w)
                    if w == NWIN - 1:
                        ag_all(i + 1)

            def wee_bcast(i):
                """WeE of block i into rows 112:128 of every vw col block."""
                nc.sync.dma_start(
                    vw[WN:128, :].rearrange("p (w d) -> p w d", w=NWIN),
                    inp[f"wee{i}"][:, None, :].to_broadcast([ED, NWIN, 128]))

            # ---- prologue: UV of block 1 from x ----
            # five UV windows in flight: pnode + 2 pexp buffers x 2 slots
            wee_bcast(1)
            pe_hold = None
            for w in range(NWIN):
                m = w % 5
                if m == 0:
                    uv_phase(1, w)
                else:
                    if m in (1, 3):
                        pe_hold = pexp.tile([128, GRP * 128], f32, tag="pe",
                                            name="puv_alt")
                        uv_phase(1, w, puv=pe_hold[:, 0:256])
                    else:
                        uv_phase(1, w, puv=pe_hold[:, 256:512])
            ag_all(1)

            # ---- pipelined blocks ----
            for i in range(1, 5):
                for w in range(PF):
                    issue_loads(i, w)
                # flat stream of group-steps with skew:
                #   exp(G) | add(G-1) | relu(G-2) | red(G-3)
                # tails for window w ride at steps (w+1, 3), (w+1, 4),
                # (w+2, 0) of the stream.
                TOT = (NWIN + 2) * NG  # flush room
                for G in range(TOT):
                    w, g = divmod(G, NG)
                    if g == 0 and w + PF < NWIN:
                        issue_loads(i, w + PF)
                    if w < NWIN:
                        exp_group(i, w, g)
                    for (dk, fn) in ((1, add_group), (2, relu_group),
                                     (3, red_group)):
                        Gp = G - dk
                        if Gp >= 0:
                            wp, gp = divmod(Gp, NG)
                            if wp < NWIN:
                                fn(i, wp, gp)
                    # staggered tails: tail1(w-1)@g3, tail2(w-1)@g4,
                    # tail3(w-2)@g0
                    if g == 3 and 0 <= w - 1 < NWIN:
                        tail1(i, w - 1)
                    if g == 4 and 0 <= w - 1 < NWIN:
                        tail2(i, w - 1)
                    if g == 0 and 0 <= w - 2 < NWIN:
                        tail3(i, w - 2)
                if i < 3:
                    wee_bcast(i + 1)

    nc.finalize()
    return nc


_NC_CACHE = {}


def kernel(**inputs):
    from concourse.bass_utils import run_bass_kernel_spmd

    in_maps = _prep_inputs(inputs)
    if "nc" not in _NC_CACHE:
        _NC_CACHE["nc"] = _build()
    nc = _NC_CACHE["nc"]
    res = run_bass_kernel_spmd(nc, in_maps, core_ids=list(range(NCORES)))
    outs = [res.results[c]["out"].reshape(-1) for c in range(NCORES)]
    return np.concatenate(outs)[:N].reshape(N, 1).astype(np.float32)


# revision 55
# speedup vs baseline: 1.0464x; 1.0294x over previous
"""Distributed Trainium2 kernel for the 4-block GNN (nn_ActorGNN).

Strategy (edge-parallel, dst-sharded), v3:
  - Pad N=100000 -> NP=100352 = 8 * 12544 nodes; core c owns nodes
    [c*12544, (c+1)*12544).  Node features live transposed in SBUF (H^T,
    bf16).
  - Algebra: the edge MLP  relu([x_src|x_dst|ea] @ We + be)  is split as
    relu(U[src] + V[dst] + ea@WeE + be) with U = x@WeS, V = x@WeD computed
    per node shard (cheap N-side matmuls).
  - U is AllGathered (fp8) in two segments (second half overlaps the tail
    windows of the previous block); every core gathers arbitrary source
    rows with SWDGE indirect DMA; V/agg stay core-local (edges live on the
    core that owns their destination).
  - Edges are grouped by destination window of 112 nodes and padded to a
    uniform 30 chunks x 128 edges per window (SPMD-uniform).  112 was
    chosen so the expand matmul's contraction packs [S^T(112) ; ea(16)]
    against [V_win(112) ; WeE(16)] - the edge-attr matmul rides along for
    free.  Per chunk PE does two matmuls: the merged expand (fp8 one-hot
    lhsT x bf16 vw) and the one-hot segment-reduce; the gathered U rows
    are folded in by the vector engine while it drains the expand PSUM to
    SBUF (bf16), and the scalar engine applies relu -> fp8.
  - The per-window dataflow is software-pipelined as a flat stream of
    chunk-groups with skew  exp(G) | add(G-1) | relu(G-2) | reduce(G-3)
    so the PE never waits for the DVE/ACT round-trip, and the per-window
    tail (scatter-mean, node update, next-block U/V) is staggered across
    the following window's group steps.
"""

import numpy as np
import ml_dtypes

BF16 = ml_dtypes.bfloat16
F8 = ml_dtypes.float8_e4m3

N = 100_000
E = 3_200_000
D = 128
ED = 16
NCORES = 8
NS = 12_544           # nodes per core
NP = NS * NCORES      # padded node count
WN = 112              # nodes per window (112 + 16 ea rows = 128 = PE K)
NWIN = NS // WN       # 112 windows per core
CHW = 30              # chunks (of 128 edges) per window, uniform
GRP = 6               # chunks per relu/expand-psum group
NG = CHW // GRP       # groups per window (5)
ES = NWIN * CHW * 128  # padded edge slots per core
WQ = 4                # windows per srcg slab load
NSEG = 2              # AllGather segments
NS2 = NS // NSEG      # AllGather segment size (nodes)
WSEG = NWIN // NSEG   # windows per AG segment
PF = 4                # windows of prefetch for edge slabs


def _seg_remap(g):
    """Map global node id -> row in the segment-ordered u_full layout."""
    r, j = g // NS, g % NS
    s, j2 = j // NS2, j % NS2
    return s * (NP // NSEG) + r * NS2 + j2


# ---------------------------------------------------------------------------
# host-side preparation
# ---------------------------------------------------------------------------

def _prep_edges(edge_index, edge_attr):
    """Distribute edges to cores/windows; build per-core slot arrays."""
    src = edge_index[0].astype(np.int64)
    dst = edge_index[1].astype(np.int64)

    cnt = np.bincount(dst, minlength=NP).astype(np.float32)
    invc_full = 1.0 / np.maximum(cnt, 1.0)

    core = dst // NS
    win = (dst % NS) // WN
    l = dst % WN

    per_core = []
    for c in range(NCORES):
        m = core == c
        s_c, w_c, l_c = src[m], win[m], l[m]
        order = np.argsort(w_c, kind="stable")
        s_c, w_c, l_c = s_c[order], w_c[order], l_c[order]
        ea_c = edge_attr[m][order]

        counts = np.bincount(w_c, minlength=NWIN)
        assert counts.max() <= CHW * 128, f"window overflow: {counts.max()}"
        starts = np.concatenate([[0], np.cumsum(counts)])

        # slot arrays, (chunk, partition) order inside each window
        srcg = np.zeros((128, NWIN * CHW), dtype=np.int32)
        lcol = np.full((128, NWIN * CHW), -1, dtype=np.int64)
        eat = np.zeros((ED, ES), dtype=np.float32)

        for w in range(NWIN):
            k = counts[w]
            sl = slice(starts[w], starts[w + 1])
            # sort window edges by src: gather descriptors walk HBM in
            # ascending address order (better DRAM page locality)
            ow = np.argsort(s_c[sl], kind="stable")
            sw, lw, ew_ = s_c[sl][ow], l_c[sl][ow], ea_c[sl][ow]
            j = np.arange(k)
            ch = w * CHW + j // 128
            p = j % 128
            srcg[p, ch] = sw
            lcol[p, ch] = lw
            eat[:, ch * 128 + p] = ew_.T

        # stk: stacked stationary [S^T(112) ; ea(16)] per edge slot, fp8
        stk = np.zeros((128, ES), dtype=F8)
        lflat = lcol.T.reshape(-1)  # slot s = g*128 + p
        pos_valid = np.nonzero(lflat >= 0)[0]
        stk[lflat[pos_valid], pos_valid] = 1.0
        stk[WN:] = eat.astype(F8)

        # ssl: reduce one-hot S per chunk [slot(128) x l(112)], fp8, packed
        # at 112-col pitch.  The reduce LDWEIGHTS reads a 128-col view that
        # overlaps 16 cols into the next chunk (junk accumulates into pw
        # rows 112:128, which are never read) so FWL still triggers.
        ssl = np.zeros((128, NWIN * CHW * 112 + 16), dtype=F8)
        pp = pos_valid % 128
        cc = pos_valid // 128
        ssl[pp, cc * 112 + lflat[pos_valid]] = 1.0

        # retile window-PAIR-major so each pair's slab is one contiguous
        # region in HBM with 2x-sized per-partition descriptors
        stk = np.ascontiguousarray(
            stk.reshape(128, NWIN // 2, 2 * CHW * 128).transpose(1, 0, 2))
        sslw = np.zeros((NWIN // 2, 128, 2 * CHW * 112 + 16), dtype=F8)
        for p in range(NWIN // 2):
            sslw[p, :, :2 * CHW * 112] = \
                ssl[:, p * 2 * CHW * 112:(p + 1) * 2 * CHW * 112]
        ssl = sslw

        per_core.append(
            dict(
                srcg=srcg.astype(np.int32),
                stk=stk,
                ssl=ssl,
                invc=invc_full[c * NS:(c + 1) * NS].reshape(NWIN, WN).T.copy(),
            )
        )
    return per_core


def _prep_inputs(inputs):
    x = inputs["x"]
    xp = np.zeros((NP, D), dtype=np.float32)
    xp[:N] = x
    per_core_edges = _prep_edges(np.asarray(inputs["edge_index"]),
                                 np.asarray(inputs["edge_attr"]))

    ones1 = np.ones((1, 128), dtype=np.float32)

    blocks = []
    for i in range(1, 5):
        We = np.asarray(inputs[f"We{i}"], np.float32)
        be = np.asarray(inputs[f"be{i}"], np.float32)
        Wn = np.asarray(inputs[f"Wn{i}"], np.float32)
        bn = np.asarray(inputs[f"bn{i}"], np.float32)
        din = We.shape[0] - ED
        din //= 2
        dout = We.shape[1]
        WeS, WeD, WeE = We[:din], We[din:2 * din], We[2 * din:]
        # pad dout -> 128
        wesd = np.zeros((128, 256), np.float32)
        wesd[:din, :dout] = WeS
        wesd[:din, 128:128 + dout] = WeD
        berow = np.zeros((1, 256), np.float32)
        berow[0, 128:128 + dout] = be
        wee = np.zeros((ED, 128), np.float32)
        wee[:, :dout] = WeE
        wnt = np.zeros((128, dout), np.float32)
        wnt[:din] = Wn[:din]
        wnb = np.zeros((128, dout), np.float32)
        wnb[:dout] = Wn[din:]
        bncol = bn.reshape(dout, 1).astype(np.float32)
        b = dict(wesd=wesd.astype(BF16), berow=berow.astype(BF16),
                 wee=wee.astype(BF16), wnt=wnt.astype(BF16),
                 wnb=wnb.astype(BF16), bn=bncol)
        if i == 4:
            # slim block-4 params: dout=1; pad the U/V pair to 64 columns so
            # the bounce rows are 128B (sub-burst HBM writes trigger RMW
            # stalls that throttled every DMA engine during block 3)
            wesd4b = np.zeros((128, 64), np.float32)
            wesd4b[:din, 0] = WeS[:, 0]
            wesd4b[:din, 1] = WeD[:, 0]
            berow4b = np.zeros((1, 64), np.float32)
            berow4b[0, 1] = be[0]
            b["wesdb"] = wesd4b.astype(BF16)
            b["berowb"] = berow4b.astype(BF16)
            b["weeb"] = np.tile(WeE[:, :1], (1, NWIN)).astype(BF16)
        blocks.append(b)

    in_maps = []
    for c in range(NCORES):
        m = dict(
            xT=xp[c * NS:(c + 1) * NS].T.astype(BF16).copy(),
            srcg=per_core_edges[c]["srcg"],
            stk=per_core_edges[c]["stk"],
            ssl=per_core_edges[c]["ssl"],
            invc=per_core_edges[c]["invc"],
            ones1=ones1.astype(BF16),
        )
        for i, b in enumerate(blocks, 1):
            for k, v in b.items():
                m[f"{k}{i}"] = v
        in_maps.append(m)
    return in_maps


# ---------------------------------------------------------------------------
# bass program
# ---------------------------------------------------------------------------

def _build():
    from concourse import bacc, bass, mybir, tile
    from concourse.masks import make_identity

    f32 = mybir.dt.float32
    bf16 = mybir.dt.bfloat16
    fp8 = mybir.dt.float8e4
    i32 = mybir.dt.int32

    nc = bacc.Bacc("TRN2", num_devices=NCORES)

    inp = {}
    for name, shape, dt in [
        ("xT", [128, NS], bf16),
        ("srcg", [128, NWIN * CHW], i32),
        ("stk", [NWIN // 2, 128, 2 * CHW * 128], fp8),
        ("ssl", [NWIN // 2, 128, 2 * CHW * 112 + 16], fp8),
        ("invc", [WN, NWIN], f32),
        ("ones1", [1, 128], bf16),
    ]:
        inp[name] = nc.dram_tensor(name, shape, dt, kind="ExternalInput")
    for i in range(1, 5):
        dout = 1 if i == 4 else D
        for name, shape, dt in [
            (f"wesd{i}", [128, 256], bf16),
            (f"berow{i}", [1, 256], bf16),
            (f"wee{i}", [ED, 128], bf16),
            (f"wnt{i}", [128, dout], bf16),
            (f"wnb{i}", [128, dout], bf16),
            (f"bn{i}", [dout, 1], f32),
        ]:
            inp[name] = nc.dram_tensor(name, shape, dt, kind="ExternalInput")
    for name, shape, dt in [
        ("wesdb4", [128, 64], bf16),
        ("berowb4", [1, 64], bf16),
        ("weeb4", [ED, NWIN], bf16),
    ]:
        inp[name] = nc.dram_tensor(name, shape, dt, kind="ExternalInput")
    out_ext = nc.dram_tensor("out", [1, NS], f32, kind="ExternalOutput")

    with tile.TileContext(nc) as tc:
        with tc.tile_pool(name="res", bufs=1) as res, \
             tc.tile_pool(name="sb", bufs=2) as sb, \
             tc.tile_pool(name="pexp", bufs=2, space="PSUM") as pexp, \
             tc.tile_pool(name="pnode", bufs=1, space="PSUM") as pnode, \
             tc.tile_pool(name="ppw", bufs=2, space="PSUM") as ppw, \
             tc.tile_pool(name="ppt", bufs=1, space="PSUM") as ppt, \
             tc.tile_pool(name="dram", bufs=1, space="DRAM") as dram:

            # resident tensors
            hT = res.tile([128, NS], bf16)         # node features, transposed
            # [V_win(112) ; WeE(16)] per window, window w at cols w*128
            vw = res.tile([128, NWIN * 128], bf16)
            vw4 = res.tile([128, NWIN], bf16)
            invc_sb = res.tile([WN, NWIN], f32)
            ones_sb = res.tile([1, 128], bf16)
            ident = res.tile([128, 128], bf16)

            nc.sync.dma_start(hT[:], inp["xT"][:])
            nc.sync.dma_start(invc_sb[:], inp["invc"][:])
            nc.sync.dma_start(ones_sb[:], inp["ones1"][:])
            nc.sync.dma_start(vw4[WN:128, :], inp["weeb4"][:])
            make_identity(nc, ident[:])

            # per-block weights, all resident
            wesd_sb, berow_sb, wee_sb, wnt_sb, wnb_sb, bn_sb = \
                {}, {}, {}, {}, {}, {}
            for i in range(1, 5):
                dout = 1 if i == 4 else D
                if i < 4:
                    wesd_sb[i] = res.tile([128, 256], bf16, tag=f"wesd{i}", name=f"wesd{i}")
                    berow_sb[i] = res.tile([1, 256], bf16, tag=f"berow{i}", name=f"berow{i}")
                    wee_sb[i] = res.tile([ED, 128], bf16, tag=f"wee{i}", name=f"wee{i}")
                    nc.sync.dma_start(wesd_sb[i][:], inp[f"wesd{i}"][:])
                    nc.sync.dma_start(berow_sb[i][:], inp[f"berow{i}"][:])
                    nc.sync.dma_start(wee_sb[i][:], inp[f"wee{i}"][:])
                else:
                    wesd_sb[i] = res.tile([128, 64], bf16, tag="wesd4", name="wesd4")
                    berow_sb[i] = res.tile([1, 64], bf16, tag="berow4", name="berow4")
                    nc.sync.dma_start(wesd_sb[i][:], inp["wesdb4"][:])
                    nc.sync.dma_start(berow_sb[i][:], inp["berowb4"][:])
                wnt_sb[i] = res.tile([128, dout], bf16, tag=f"wnt{i}", name=f"wnt{i}")
                wnb_sb[i] = res.tile([128, dout], bf16, tag=f"wnb{i}", name=f"wnb{i}")
                bn_sb[i] = res.tile([dout, 1], f32, tag=f"bn{i}", name=f"bn{i}")
                nc.sync.dma_start(wnt_sb[i][:], inp[f"wnt{i}"][:])
                nc.sync.dma_start(wnb_sb[i][:], inp[f"wnb{i}"][:])
                nc.sync.dma_start(bn_sb[i][:], inp[f"bn{i}"][:])

            u_bounce = dram.tile([NS, 128], fp8)
            u4_bounce = dram.tile([NS, 64], bf16, name="u4_bounce",
                                  tag="u4_bounce")
            u4_pack = dram.tile([NS, 2], bf16, name="u4_pack",
                                tag="u4_pack")

            u_fulls = {}
            for i in (3, 2, 1):  # reversed alloc order (block-3 DMA probe)
                uf = dram.tile([NP, 128], fp8, addr_space="Shared",
                               name=f"u_full{i}", tag=f"u_full{i}")
                u_fulls[i] = uf
            u4f = dram.tile([NP, 2], bf16, addr_space="Shared",
                            name="u4_full", tag="u4_full")
            u_fulls[4] = u4f

            def uv_phase(i, w, puv=None):
                """Compute U/V of block i for window w from current hT."""
                slim = i == 4
                wc = slice(w * WN, (w + 1) * WN)
                if puv is None:
                    puv = pnode.tile([128, 256], f32, tag="pnode",
                                     name="puv")
                nuv = 64 if slim else 256
                nc.tensor.matmul(out=puv[:WN, :nuv],
                                 lhsT=hT[:, wc], rhs=wesd_sb[i][:],
                                 start=True, stop=False)
                nc.tensor.matmul(out=puv[:WN, :nuv],
                                 lhsT=ones_sb[:, :WN],
                                 rhs=berow_sb[i][:], start=False, stop=True)
                if not slim:
                    utile = sb.tile([WN, 128], fp8, tag="utile")
                    nc.scalar.copy(utile[:], puv[:WN, :128])
                    nc.vector.tensor_copy(vw[:WN, w * 128:(w + 1) * 128],
                                          puv[:WN, 128:256])
                    nc.sync.dma_start(u_bounce[wc, :], utile[:])
                else:
                    utile = sb.tile([WN, 64], bf16, tag="utile4")
                    nc.scalar.copy(utile[:], puv[:WN, :64])
                    nc.vector.tensor_copy(vw4[:WN, w:w + 1], puv[:WN, 1:2])
                    nc.sync.dma_start(u4_bounce[wc, :], utile[:])

            def ag_all(i):
                """AllGather block i's U into u_fulls[i]."""
                full = u_fulls[i]
                if i == 4:
                    # repack the 128B-row bounce into the compact layout the
                    # AllGather/gather expects.  Round-trip through SBUF so
                    # the DRAM write side is 392B-contiguous per partition
                    # (a direct strided copy emits 12544 4-byte writes that
                    # each trigger an HBM read-modify-write)
                    u4sb = sb.tile([128, (NS // 128) * 2], bf16,
                                   tag="u4sb", name="u4sb")
                    nc.sync.dma_start(
                        u4sb[:].rearrange("p (k t) -> p k t", t=2),
                        u4_bounce[:, 0:2].rearrange("(p k) t -> p k t",
                                                    p=128))
                    nc.sync.dma_start(
                        u4_pack[:].rearrange("(p k) t -> p k t", p=128),
                        u4sb[:].rearrange("p (k t) -> p k t", t=2))
                    in_ap = u4_pack[:]
                else:
                    in_ap = u_bounce[:]
                nc.gpsimd.collective_compute(
                    "AllGather", mybir.AluOpType.bypass,
                    replica_groups=[list(range(NCORES))],
                    ins=[in_ap.opt()],
                    outs=[full.opt()],
                )

            # ------------------------------------------------------------
            # pipelined block body
            # ------------------------------------------------------------
            state = {}

            def issue_loads(i, w):
                """Prefetch edge slabs for window w of block i."""
                slim = i == 4
                if w % WQ == 0:
                    srcg_sl = sb.tile([128, WQ * CHW], i32, tag="srcg",
                                      bufs=3)
                    nc.sync.dma_start(
                        srcg_sl[:],
                        inp["srcg"][:, w * CHW:(w + WQ) * CHW])
                    state["srcg"] = srcg_sl
                w0 = (w % WQ) * CHW
                uslab = sb.tile(
                    [128, CHW * 2] if slim else [128, CHW * 128],
                    bf16 if slim else fp8,
                    tag="uslab4" if slim else "uslab", bufs=PF + 2)
                nc.gpsimd.indirect_dma_start(
                    out=uslab[:],
                    out_offset=None,
                    in_=u_fulls[i][:],
                    in_offset=bass.IndirectOffsetOnAxis(
                        ap=state["srcg"][:, w0:w0 + CHW], axis=0),
                )
                if w % 2 == 0:
                    stslab = sb.tile([128, 2 * CHW * 128], fp8, tag="stslab",
                                     bufs=PF // 2 + 1)
                    nc.sync.dma_start(stslab[:], inp["stk"][w // 2])
                    sslab = sb.tile([128, 2 * CHW * 112 + 16], fp8,
                                    tag="sslab", bufs=PF // 2 + 2)
                    nc.sync.dma_start(sslab[:], inp["ssl"][w // 2])
                    state[("pair", w // 2)] = (stslab, sslab)
                state[("sl", w)] = (uslab,) + state[("pair", w // 2)]

            def exp_group(i, w, g):
                slim = i == 4
                _, stslab, _ = state[("sl", w)]
                g0 = g * GRP
                sb0 = (w % 2) * CHW * 128
                if not slim:
                    pe_ = pexp.tile([128, GRP * 128], f32, tag="pe")
                    for c in range(g0, g0 + GRP):
                        r = (c - g0) * 128
                        nc.tensor.matmul(
                            out=pe_[:, r:r + 128],
                            lhsT=stslab[:, sb0 + c * 128:sb0 + (c + 1) * 128],
                            rhs=vw[:, w * 128:(w + 1) * 128],
                            start=True, stop=True)
                else:
                    pe_ = pexp.tile([128, GRP], f32, tag="pe")
                    for c in range(g0, g0 + GRP):
                        nc.tensor.matmul(
                            out=pe_[:, c - g0:c - g0 + 1],
                            lhsT=stslab[:, sb0 + c * 128:sb0 + (c + 1) * 128],
                            rhs=vw4[:, w:w + 1],
                            start=True, stop=True)
                state[("pe", w, g)] = pe_

            def add_group(i, w, g):
                """Drain expand PSUM + add gathered U rows -> SBUF bf16."""
                slim = i == 4
                uslab, _, _ = state[("sl", w)]
                pe_ = state.pop(("pe", w, g))
                g0 = g * GRP
                if not slim:
                    smt = sb.tile([128, GRP * 128], bf16, tag="smt", bufs=4)
                    nc.vector.tensor_tensor(
                        out=smt[:], in0=pe_[:],
                        in1=uslab[:, g0 * 128:(g0 + GRP) * 128],
                        op=mybir.AluOpType.add)
                else:
                    smt = sb.tile([128, GRP], bf16, tag="smt4", bufs=4)
                    u4r = uslab[:].rearrange("p (c t) -> p c t", t=2)
                    nc.vector.tensor_tensor(
                        out=smt[:], in0=pe_[:],
                        in1=u4r[:, g0:g0 + GRP, 0:1],
                        op=mybir.AluOpType.add)
                state[("sm", w, g)] = smt

            def relu_group(i, w, g):
                slim = i == 4
                smt = state.pop(("sm", w, g))
                if not slim:
                    wslab = sb.tile([128, GRP * 128], bf16, tag="wslab",
                                    bufs=4)
                else:
                    wslab = sb.tile([128, GRP], bf16, tag="wslab4", bufs=4)
                if g == NG - 1 and not slim:
                    # one group per window on DVE (4x mode, 16-bit SBUF)
                    nc.vector.tensor_scalar(
                        out=wslab[:], in0=smt[:], scalar1=0.0, scalar2=None,
                        op0=mybir.AluOpType.max)
                else:
                    nc.scalar.activation(
                        wslab[:], smt[:], mybir.ActivationFunctionType.Relu)
                state[("ws", w, g)] = wslab

            def red_group(i, w, g):
                slim = i == 4
                _, _, sslab = state[("sl", w)]
                wslab = state.pop(("ws", w, g))
                if g == 0:
                    state[("pw", w)] = ppw.tile([128, 128], f32, tag="pw",
                                                name="pw")
                pw = state[("pw", w)]
                g0 = g * GRP
                rb0 = (w % 2) * CHW * 112
                for c in range(g0, g0 + GRP):
                    cc = c - g0
                    if not slim:
                        nc.tensor.matmul(
                            out=pw[:, :],
                            lhsT=sslab[:, rb0 + c * 112:rb0 + c * 112 + 128],
                            rhs=wslab[:, cc * 128:(cc + 1) * 128],
                            start=(c == 0), stop=(c == CHW - 1))
                    else:
                        nc.tensor.matmul(
                            out=pw[:, :1],
                            lhsT=sslab[:, rb0 + c * 112:rb0 + c * 112 + 128],
                            rhs=wslab[:, cc:cc + 1],
                            start=(c == 0), stop=(c == CHW - 1))
                if g == NG - 1:
                    state.pop(("sl", w))
                    if w % 2 == 1:
                        state.pop(("pair", w // 2))

            def tail1(i, w):
                """scatter-mean scale + transpose."""
                slim = i == 4
                nd = 1 if slim else 128
                pw = state.pop(("pw", w))
                argm = sb.tile([WN, nd], bf16,
                               tag="argm4" if slim else "argm", bufs=2)
                # drain + scatter-mean scale in one ACT op (per-partition
                # scale port carries 1/max(cnt,1))
                nc.scalar.activation(
                    argm[:], pw[:WN, :nd],
                    mybir.ActivationFunctionType.Identity,
                    scale=invc_sb[:, w:w + 1])
                pt = ppt.tile([nd, WN], bf16, tag="pt")
                nc.tensor.transpose(pt[:], argm[:], ident[:WN, :WN])
                state[("pt", w)] = pt

            def tail2(i, w):
                """aggregate -> node update -> new hT (or sigmoid out)."""
                slim = i == 4
                dout = 1 if slim else D
                nd = 1 if slim else 128
                pt = state.pop(("pt", w))
                aggt = sb.tile([128, WN], bf16, tag="aggt", bufs=2)
                nc.scalar.copy(aggt[:nd, :], pt[:])
                wc = slice(w * WN, (w + 1) * WN)
                pupd = pnode.tile([128, 128], f32, tag="pnode")
                nc.tensor.matmul(out=pupd[:dout, :WN], lhsT=wnt_sb[i][:],
                                 rhs=hT[:, wc], start=True, stop=False)
                nc.tensor.matmul(out=pupd[:dout, :WN], lhsT=wnb_sb[i][:],
                                 rhs=aggt[:], start=False, stop=True)
                if not slim:
                    nc.scalar.activation(
                        hT[:, wc], pupd[:, :WN],
                        mybir.ActivationFunctionType.Relu,
                        bias=bn_sb[i][:])
                else:
                    out_t = sb.tile([1, WN], f32, tag="out_t")
                    nc.scalar.activation(
                        out_t[:], pupd[:1, :WN],
                        mybir.ActivationFunctionType.Sigmoid,
                        bias=bn_sb[i][:])
                    nc.sync.dma_start(out_ext[:, wc], out_t[:])

            def tail3(i, w):
                """U/V of block i+1 for window w + segment AllGathers."""
                if i < 4:
                    uv_phase(i + 1, w)
                    if w == NWIN - 1:
                        ag_all(i + 1)

            def wee_bcast(i):
                """WeE of block i into rows 112:128 of every vw col block."""
                nc.sync.dma_start(
                    vw[WN:128, :].rearrange("p (w d) -> p w d", w=NWIN),
                    inp[f"wee{i}"][:, None, :].to_broadcast([ED, NWIN, 128]))

            # ---- prologue: UV of block 1 from x ----
            # five UV windows in flight: pnode + 2 pexp buffers x 2 slots
            wee_bcast(1)
            pe_hold = None
            for w in range(NWIN):
                m = w % 5
                if m == 0:
                    uv_phase(1, w)
                else:
                    if m in (1, 3):
                        pe_hold = pexp.tile([128, GRP * 128], f32, tag="pe",
                                            name="puv_alt")
                        uv_phase(1, w, puv=pe_hold[:, 0:256])
                    else:
                        uv_phase(1, w, puv=pe_hold[:, 256:512])
            ag_all(1)

            # ---- pipelined blocks ----
            for i in range(1, 5):
                for w in range(PF):
                    issue_loads(i, w)
                # flat stream of group-steps with skew:
                #   exp(G) | add(G-1) | relu(G-2) | red(G-3)
                # tails for window w ride at steps (w+1, 3), (w+1, 4),
                # (w+2, 0) of the stream.
                TOT = (NWIN + 2) * NG  # flush room
                for G in range(TOT):
                    w, g = divmod(G, NG)
                    if g == 0 and w + PF < NWIN:
                        issue_loads(i, w + PF)
                    if w < NWIN:
                        exp_group(i, w, g)
                    for (dk, fn) in ((1, add_group), (2, relu_group),
                                     (3, red_group)):
                        Gp = G - dk
                        if Gp >= 0:
                            wp, gp = divmod(Gp, NG)
                            if wp < NWIN:
                                fn(i, wp, gp)
                    # staggered tails: tail1(w-1)@g3, tail2(w-1)@g4,
                    # tail3(w-2)@g0
                    if g == 3 and 0 <= w - 1 < NWIN:
                        tail1(i, w - 1)
                    if g == 4 and 0 <= w - 1 < NWIN:
                        tail2(i, w - 1)
                    if g == 0 and 0 <= w - 2 < NWIN:
                        tail3(i, w - 2)
                if i < 3:
                    wee_bcast(i + 1)

    nc.finalize()
    return nc


_NC_CACHE = {}


def kernel(**inputs):
    from concourse.bass_utils import run_bass_kernel_spmd

    in_maps = _prep_inputs(inputs)
    if "nc" not in _NC_CACHE:
        _NC_CACHE["nc"] = _build()
    nc = _NC_CACHE["nc"]
    res = run_bass_kernel_spmd(nc, in_maps, core_ids=list(range(NCORES)))
    outs = [res.results[c]["out"].reshape(-1) for c in range(NCORES)]
    return np.concatenate(outs)[:N].reshape(N, 1).astype(np.float32)


# revision 56
# speedup vs baseline: 1.0779x; 1.0301x over previous
"""Distributed Trainium2 kernel for the 4-block GNN (nn_ActorGNN).

Strategy (edge-parallel, dst-sharded), v3:
  - Pad N=100000 -> NP=100352 = 8 * 12544 nodes; core c owns nodes
    [c*12544, (c+1)*12544).  Node features live transposed in SBUF (H^T,
    bf16).
  - Algebra: the edge MLP  relu([x_src|x_dst|ea] @ We + be)  is split as
    relu(U[src] + V[dst] + ea@WeE + be) with U = x@WeS, V = x@WeD computed
    per node shard (cheap N-side matmuls).
  - U is AllGathered (fp8) in two segments (second half overlaps the tail
    windows of the previous block); every core gathers arbitrary source
    rows with SWDGE indirect DMA; V/agg stay core-local (edges live on the
    core that owns their destination).
  - Edges are grouped by destination window of 112 nodes and padded to a
    uniform 30 chunks x 128 edges per window (SPMD-uniform).  112 was
    chosen so the expand matmul's contraction packs [S^T(112) ; ea(16)]
    against [V_win(112) ; WeE(16)] - the edge-attr matmul rides along for
    free.  Per chunk PE does two matmuls: the merged expand (fp8 one-hot
    lhsT x bf16 vw) and the one-hot segment-reduce; the gathered U rows
    are folded in by the vector engine while it drains the expand PSUM to
    SBUF (bf16), and the scalar engine applies relu -> fp8.
  - The per-window dataflow is software-pipelined as a flat stream of
    chunk-groups with skew  exp(G) | add(G-1) | relu(G-2) | reduce(G-3)
    so the PE never waits for the DVE/ACT round-trip, and the per-window
    tail (scatter-mean, node update, next-block U/V) is staggered across
    the following window's group steps.
"""

import numpy as np
import ml_dtypes

BF16 = ml_dtypes.bfloat16
F8 = ml_dtypes.float8_e4m3

N = 100_000
E = 3_200_000
D = 128
ED = 16
NCORES = 8
NS = 12_544           # nodes per core
NP = NS * NCORES      # padded node count
WN = 112              # nodes per window (112 + 16 ea rows = 128 = PE K)
NWIN = NS // WN       # 112 windows per core
CHW = 30              # chunks (of 128 edges) per window, uniform
GRP = 6               # chunks per relu/expand-psum group
NG = CHW // GRP       # groups per window (5)
ES = NWIN * CHW * 128  # padded edge slots per core
WQ = 4                # windows per srcg slab load
NSEG = 2              # AllGather segments
NS2 = NS // NSEG      # AllGather segment size (nodes)
WSEG = NWIN // NSEG   # windows per AG segment
PF = 4                # windows of prefetch for edge slabs


def _seg_remap(g):
    """Map global node id -> row in the segment-ordered u_full layout."""
    r, j = g // NS, g % NS
    s, j2 = j // NS2, j % NS2
    return s * (NP // NSEG) + r * NS2 + j2


# ---------------------------------------------------------------------------
# host-side preparation
# ---------------------------------------------------------------------------

def _prep_edges(edge_index, edge_attr):
    """Distribute edges to cores/windows; build per-core slot arrays."""
    src = edge_index[0].astype(np.int64)
    dst = edge_index[1].astype(np.int64)

    cnt = np.bincount(dst, minlength=NP).astype(np.float32)
    invc_full = 1.0 / np.maximum(cnt, 1.0)

    core = dst // NS
    win = (dst % NS) // WN
    l = dst % WN

    per_core = []
    for c in range(NCORES):
        m = core == c
        s_c, w_c, l_c = src[m], win[m], l[m]
        order = np.argsort(w_c, kind="stable")
        s_c, w_c, l_c = s_c[order], w_c[order], l_c[order]
        ea_c = edge_attr[m][order]

        counts = np.bincount(w_c, minlength=NWIN)
        assert counts.max() <= CHW * 128, f"window overflow: {counts.max()}"
        starts = np.concatenate([[0], np.cumsum(counts)])

        # slot arrays, (chunk, partition) order inside each window
        srcg = np.zeros((128, NWIN * CHW), dtype=np.int32)
        lcol = np.full((128, NWIN * CHW), -1, dtype=np.int64)
        eat = np.zeros((ED, ES), dtype=np.float32)

        for w in range(NWIN):
            k = counts[w]
            sl = slice(starts[w], starts[w + 1])
            # sort window edges by src: gather descriptors walk HBM in
            # ascending address order (better DRAM page locality)
            ow = np.argsort(s_c[sl], kind="stable")
            sw, lw, ew_ = s_c[sl][ow], l_c[sl][ow], ea_c[sl][ow]
            j = np.arange(k)
            ch = w * CHW + j // 128
            p = j % 128
            srcg[p, ch] = sw
            lcol[p, ch] = lw
            eat[:, ch * 128 + p] = ew_.T

        # stk: stacked stationary [S^T(112) ; ea(16)] per edge slot, fp8
        stk = np.zeros((128, ES), dtype=F8)
        lflat = lcol.T.reshape(-1)  # slot s = g*128 + p
        pos_valid = np.nonzero(lflat >= 0)[0]
        stk[lflat[pos_valid], pos_valid] = 1.0
        stk[WN:] = eat.astype(F8)

        # ssl: reduce one-hot S per chunk [slot(128) x l(112)], fp8, packed
        # at 112-col pitch.  The reduce LDWEIGHTS reads a 128-col view that
        # overlaps 16 cols into the next chunk (junk accumulates into pw
        # rows 112:128, which are never read) so FWL still triggers.
        ssl = np.zeros((128, NWIN * CHW * 112 + 16), dtype=F8)
        pp = pos_valid % 128
        cc = pos_valid // 128
        ssl[pp, cc * 112 + lflat[pos_valid]] = 1.0

        # retile window-PAIR-major so each pair's slab is one contiguous
        # region in HBM with 2x-sized per-partition descriptors
        stk = np.ascontiguousarray(
            stk.reshape(128, NWIN // 2, 2 * CHW * 128).transpose(1, 0, 2))
        sslw = np.zeros((NWIN // 2, 128, 2 * CHW * 112 + 16), dtype=F8)
        for p in range(NWIN // 2):
            sslw[p, :, :2 * CHW * 112] = \
                ssl[:, p * 2 * CHW * 112:(p + 1) * 2 * CHW * 112]
        ssl = sslw

        per_core.append(
            dict(
                srcg=srcg.astype(np.int32),
                stk=stk,
                ssl=ssl,
                invc=invc_full[c * NS:(c + 1) * NS].reshape(NWIN, WN).T.copy(),
            )
        )
    return per_core


def _prep_inputs(inputs):
    x = inputs["x"]
    xp = np.zeros((NP, D), dtype=np.float32)
    xp[:N] = x
    per_core_edges = _prep_edges(np.asarray(inputs["edge_index"]),
                                 np.asarray(inputs["edge_attr"]))

    ones1 = np.ones((1, 128), dtype=np.float32)

    blocks = []
    for i in range(1, 5):
        We = np.asarray(inputs[f"We{i}"], np.float32)
        be = np.asarray(inputs[f"be{i}"], np.float32)
        Wn = np.asarray(inputs[f"Wn{i}"], np.float32)
        bn = np.asarray(inputs[f"bn{i}"], np.float32)
        din = We.shape[0] - ED
        din //= 2
        dout = We.shape[1]
        WeS, WeD, WeE = We[:din], We[din:2 * din], We[2 * din:]
        # pad dout -> 128
        wesd = np.zeros((128, 256), np.float32)
        wesd[:din, :dout] = WeS
        wesd[:din, 128:128 + dout] = WeD
        berow = np.zeros((1, 256), np.float32)
        berow[0, 128:128 + dout] = be
        wee = np.zeros((ED, 128), np.float32)
        wee[:, :dout] = WeE
        wnt = np.zeros((128, dout), np.float32)
        wnt[:din] = Wn[:din]
        wnb = np.zeros((128, dout), np.float32)
        wnb[:dout] = Wn[din:]
        bncol = bn.reshape(dout, 1).astype(np.float32)
        b = dict(wesd=wesd.astype(BF16), berow=berow.astype(BF16),
                 wee=wee.astype(BF16), wnt=wnt.astype(BF16),
                 wnb=wnb.astype(BF16), bn=bncol)
        if i == 4:
            # slim block-4 params: dout=1; pad the U/V pair to 64 columns so
            # the bounce rows are 128B (sub-burst HBM writes trigger RMW
            # stalls that throttled every DMA engine during block 3)
            wesd4b = np.zeros((128, 64), np.float32)
            wesd4b[:din, 0] = WeS[:, 0]
            wesd4b[:din, 1] = WeD[:, 0]
            berow4b = np.zeros((1, 64), np.float32)
            berow4b[0, 1] = be[0]
            b["wesdb"] = wesd4b.astype(BF16)
            b["berowb"] = berow4b.astype(BF16)
            b["weeb"] = np.tile(WeE[:, :1], (1, NWIN)).astype(BF16)
        blocks.append(b)

    in_maps = []
    for c in range(NCORES):
        m = dict(
            xT=xp[c * NS:(c + 1) * NS].T.astype(BF16).copy(),
            srcg=per_core_edges[c]["srcg"],
            stk=per_core_edges[c]["stk"],
            ssl=per_core_edges[c]["ssl"],
            invc=per_core_edges[c]["invc"],
            ones1=ones1.astype(BF16),
        )
        for i, b in enumerate(blocks, 1):
            for k, v in b.items():
                m[f"{k}{i}"] = v
        in_maps.append(m)
    return in_maps


# ---------------------------------------------------------------------------
# bass program
# ---------------------------------------------------------------------------

def _build():
    from concourse import bacc, bass, mybir, tile
    from concourse.masks import make_identity

    f32 = mybir.dt.float32
    bf16 = mybir.dt.bfloat16
    fp8 = mybir.dt.float8e4
    i32 = mybir.dt.int32

    nc = bacc.Bacc("TRN2", num_devices=NCORES)

    inp = {}
    for name, shape, dt in [
        ("xT", [128, NS], bf16),
        ("srcg", [128, NWIN * CHW], i32),
        ("stk", [NWIN // 2, 128, 2 * CHW * 128], fp8),
        ("ssl", [NWIN // 2, 128, 2 * CHW * 112 + 16], fp8),
        ("invc", [WN, NWIN], f32),
        ("ones1", [1, 128], bf16),
    ]:
        inp[name] = nc.dram_tensor(name, shape, dt, kind="ExternalInput")
    for i in range(1, 5):
        dout = 1 if i == 4 else D
        for name, shape, dt in [
            (f"wesd{i}", [128, 256], bf16),
            (f"berow{i}", [1, 256], bf16),
            (f"wee{i}", [ED, 128], bf16),
            (f"wnt{i}", [128, dout], bf16),
            (f"wnb{i}", [128, dout], bf16),
            (f"bn{i}", [dout, 1], f32),
        ]:
            inp[name] = nc.dram_tensor(name, shape, dt, kind="ExternalInput")
    for name, shape, dt in [
        ("wesdb4", [128, 64], bf16),
        ("berowb4", [1, 64], bf16),
        ("weeb4", [ED, NWIN], bf16),
    ]:
        inp[name] = nc.dram_tensor(name, shape, dt, kind="ExternalInput")
    out_ext = nc.dram_tensor("out", [1, NS], f32, kind="ExternalOutput")

    with tile.TileContext(nc) as tc:
        with tc.tile_pool(name="res", bufs=1) as res, \
             tc.tile_pool(name="sb", bufs=2) as sb, \
             tc.tile_pool(name="pexp", bufs=2, space="PSUM") as pexp, \
             tc.tile_pool(name="pnode", bufs=1, space="PSUM") as pnode, \
             tc.tile_pool(name="ppw", bufs=2, space="PSUM") as ppw, \
             tc.tile_pool(name="ppt", bufs=1, space="PSUM") as ppt, \
             tc.tile_pool(name="dram", bufs=1, space="DRAM") as dram:

            # resident tensors
            hT = res.tile([128, NS], bf16)         # node features, transposed
            # [V_win(112) ; WeE(16)] per window, window w at cols w*128
            vw = res.tile([128, NWIN * 128], bf16)
            vw4 = res.tile([128, NWIN], bf16)
            invc_sb = res.tile([WN, NWIN], f32)
            ones_sb = res.tile([1, 128], bf16)
            ident = res.tile([128, 128], bf16)

            nc.sync.dma_start(hT[:], inp["xT"][:])
            nc.sync.dma_start(invc_sb[:], inp["invc"][:])
            nc.sync.dma_start(ones_sb[:], inp["ones1"][:])
            nc.sync.dma_start(vw4[WN:128, :], inp["weeb4"][:])
            make_identity(nc, ident[:])

            # per-block weights, all resident
            wesd_sb, berow_sb, wee_sb, wnt_sb, wnb_sb, bn_sb = \
                {}, {}, {}, {}, {}, {}
            for i in range(1, 5):
                dout = 1 if i == 4 else D
                if i < 4:
                    wesd_sb[i] = res.tile([128, 256], bf16, tag=f"wesd{i}", name=f"wesd{i}")
                    berow_sb[i] = res.tile([1, 256], bf16, tag=f"berow{i}", name=f"berow{i}")
                    wee_sb[i] = res.tile([ED, 128], bf16, tag=f"wee{i}", name=f"wee{i}")
                    nc.sync.dma_start(wesd_sb[i][:], inp[f"wesd{i}"][:])
                    nc.sync.dma_start(berow_sb[i][:], inp[f"berow{i}"][:])
                    nc.sync.dma_start(wee_sb[i][:], inp[f"wee{i}"][:])
                else:
                    wesd_sb[i] = res.tile([128, 64], bf16, tag="wesd4", name="wesd4")
                    berow_sb[i] = res.tile([1, 64], bf16, tag="berow4", name="berow4")
                    nc.sync.dma_start(wesd_sb[i][:], inp["wesdb4"][:])
                    nc.sync.dma_start(berow_sb[i][:], inp["berowb4"][:])
                wnt_sb[i] = res.tile([128, dout], bf16, tag=f"wnt{i}", name=f"wnt{i}")
                wnb_sb[i] = res.tile([128, dout], bf16, tag=f"wnb{i}", name=f"wnb{i}")
                bn_sb[i] = res.tile([dout, 1], f32, tag=f"bn{i}", name=f"bn{i}")
                nc.sync.dma_start(wnt_sb[i][:], inp[f"wnt{i}"][:])
                nc.sync.dma_start(wnb_sb[i][:], inp[f"wnb{i}"][:])
                nc.sync.dma_start(bn_sb[i][:], inp[f"bn{i}"][:])

            u_bounce = dram.tile([NS, 128], fp8)
            u4_bounce = dram.tile([NS, 64], bf16, name="u4_bounce",
                                  tag="u4_bounce")
            u4_pack = dram.tile([NS, 2], bf16, name="u4_pack",
                                tag="u4_pack")

            u_fulls = {}
            for i in (3, 2, 1):  # reversed alloc order (block-3 DMA probe)
                uf = dram.tile([NP, 128], fp8, addr_space="Shared",
                               name=f"u_full{i}", tag=f"u_full{i}")
                u_fulls[i] = uf
            u4f = dram.tile([NP, 2], bf16, addr_space="Shared",
                            name="u4_full", tag="u4_full")
            u_fulls[4] = u4f

            def uv_phase(i, w, puv=None):
                """Compute U/V of block i for window w from current hT."""
                slim = i == 4
                wc = slice(w * WN, (w + 1) * WN)
                if puv is None:
                    puv = pnode.tile([128, 256], f32, tag="pnode",
                                     name="puv")
                nuv = 64 if slim else 256
                nc.tensor.matmul(out=puv[:WN, :nuv],
                                 lhsT=hT[:, wc], rhs=wesd_sb[i][:],
                                 start=True, stop=False)
                nc.tensor.matmul(out=puv[:WN, :nuv],
                                 lhsT=ones_sb[:, :WN],
                                 rhs=berow_sb[i][:], start=False, stop=True)
                if not slim:
                    utile = sb.tile([WN, 128], fp8, tag="utile", bufs=6)
                    nc.scalar.copy(utile[:], puv[:WN, :128])
                    nc.vector.tensor_copy(vw[:WN, w * 128:(w + 1) * 128],
                                          puv[:WN, 128:256])
                    nc.sync.dma_start(u_bounce[wc, :], utile[:])
                else:
                    utile = sb.tile([WN, 64], bf16, tag="utile4", bufs=6)
                    nc.scalar.copy(utile[:], puv[:WN, :64])
                    nc.vector.tensor_copy(vw4[:WN, w:w + 1], puv[:WN, 1:2])
                    nc.sync.dma_start(u4_bounce[wc, :], utile[:])

            def ag_all(i):
                """AllGather block i's U into u_fulls[i]."""
                full = u_fulls[i]
                if i == 4:
                    # repack the 128B-row bounce into the compact layout the
                    # AllGather/gather expects.  Round-trip through SBUF so
                    # the DRAM write side is 392B-contiguous per partition
                    # (a direct strided copy emits 12544 4-byte writes that
                    # each trigger an HBM read-modify-write)
                    u4sb = sb.tile([128, (NS // 128) * 2], bf16,
                                   tag="u4sb", name="u4sb")
                    nc.sync.dma_start(
                        u4sb[:].rearrange("p (k t) -> p k t", t=2),
                        u4_bounce[:, 0:2].rearrange("(p k) t -> p k t",
                                                    p=128))
                    nc.sync.dma_start(
                        u4_pack[:].rearrange("(p k) t -> p k t", p=128),
                        u4sb[:].rearrange("p (k t) -> p k t", t=2))
                    in_ap = u4_pack[:]
                else:
                    in_ap = u_bounce[:]
                nc.gpsimd.collective_compute(
                    "AllGather", mybir.AluOpType.bypass,
                    replica_groups=[list(range(NCORES))],
                    ins=[in_ap.opt()],
                    outs=[full.opt()],
                )

            # ------------------------------------------------------------
            # pipelined block body
            # ------------------------------------------------------------
            state = {}

            def issue_loads(i, w):
                """Prefetch edge slabs for window w of block i."""
                slim = i == 4
                if w % WQ == 0:
                    srcg_sl = sb.tile([128, WQ * CHW], i32, tag="srcg",
                                      bufs=3)
                    nc.sync.dma_start(
                        srcg_sl[:],
                        inp["srcg"][:, w * CHW:(w + WQ) * CHW])
                    state["srcg"] = srcg_sl
                w0 = (w % WQ) * CHW
                uslab = sb.tile(
                    [128, CHW * 2] if slim else [128, CHW * 128],
                    bf16 if slim else fp8,
                    tag="uslab4" if slim else "uslab", bufs=PF + 2)
                nc.gpsimd.indirect_dma_start(
                    out=uslab[:],
                    out_offset=None,
                    in_=u_fulls[i][:],
                    in_offset=bass.IndirectOffsetOnAxis(
                        ap=state["srcg"][:, w0:w0 + CHW], axis=0),
                )
                if w % 2 == 0:
                    stslab = sb.tile([128, 2 * CHW * 128], fp8, tag="stslab",
                                     bufs=PF // 2 + 1)
                    nc.sync.dma_start(stslab[:], inp["stk"][w // 2])
                    sslab = sb.tile([128, 2 * CHW * 112 + 16], fp8,
                                    tag="sslab", bufs=PF // 2 + 2)
                    nc.sync.dma_start(sslab[:], inp["ssl"][w // 2])
                    state[("pair", w // 2)] = (stslab, sslab)
                state[("sl", w)] = (uslab,) + state[("pair", w // 2)]

            def exp_group(i, w, g):
                slim = i == 4
                _, stslab, _ = state[("sl", w)]
                g0 = g * GRP
                sb0 = (w % 2) * CHW * 128
                if not slim:
                    pe_ = pexp.tile([128, GRP * 128], f32, tag="pe")
                    for c in range(g0, g0 + GRP):
                        r = (c - g0) * 128
                        nc.tensor.matmul(
                            out=pe_[:, r:r + 128],
                            lhsT=stslab[:, sb0 + c * 128:sb0 + (c + 1) * 128],
                            rhs=vw[:, w * 128:(w + 1) * 128],
                            start=True, stop=True)
                else:
                    pe_ = pexp.tile([128, GRP], f32, tag="pe")
                    for c in range(g0, g0 + GRP):
                        nc.tensor.matmul(
                            out=pe_[:, c - g0:c - g0 + 1],
                            lhsT=stslab[:, sb0 + c * 128:sb0 + (c + 1) * 128],
                            rhs=vw4[:, w:w + 1],
                            start=True, stop=True)
                state[("pe", w, g)] = pe_

            def add_group(i, w, g):
                """Drain expand PSUM + add gathered U rows -> SBUF bf16."""
                slim = i == 4
                uslab, _, _ = state[("sl", w)]
                pe_ = state.pop(("pe", w, g))
                g0 = g * GRP
                if not slim:
                    smt = sb.tile([128, GRP * 128], bf16, tag="smt", bufs=4)
                    nc.vector.tensor_tensor(
                        out=smt[:], in0=pe_[:],
                        in1=uslab[:, g0 * 128:(g0 + GRP) * 128],
                        op=mybir.AluOpType.add)
                else:
                    smt = sb.tile([128, GRP], bf16, tag="smt4", bufs=4)
                    u4r = uslab[:].rearrange("p (c t) -> p c t", t=2)
                    nc.vector.tensor_tensor(
                        out=smt[:], in0=pe_[:],
                        in1=u4r[:, g0:g0 + GRP, 0:1],
                        op=mybir.AluOpType.add)
                state[("sm", w, g)] = smt

            def relu_group(i, w, g):
                slim = i == 4
                smt = state.pop(("sm", w, g))
                if not slim:
                    wslab = sb.tile([128, GRP * 128], bf16, tag="wslab",
                                    bufs=4)
                else:
                    wslab = sb.tile([128, GRP], bf16, tag="wslab4", bufs=4)
                if g == NG - 1 and not slim:
                    # one group per window on DVE (4x mode, 16-bit SBUF)
                    nc.vector.tensor_scalar(
                        out=wslab[:], in0=smt[:], scalar1=0.0, scalar2=None,
                        op0=mybir.AluOpType.max)
                else:
                    nc.scalar.activation(
                        wslab[:], smt[:], mybir.ActivationFunctionType.Relu)
                state[("ws", w, g)] = wslab

            def red_group(i, w, g):
                slim = i == 4
                _, _, sslab = state[("sl", w)]
                wslab = state.pop(("ws", w, g))
                if g == 0:
                    state[("pw", w)] = ppw.tile([128, 128], f32, tag="pw",
                                                name="pw")
                pw = state[("pw", w)]
                g0 = g * GRP
                rb0 = (w % 2) * CHW * 112
                for c in range(g0, g0 + GRP):
                    cc = c - g0
                    if not slim:
                        nc.tensor.matmul(
                            out=pw[:, :],
                            lhsT=sslab[:, rb0 + c * 112:rb0 + c * 112 + 128],
                            rhs=wslab[:, cc * 128:(cc + 1) * 128],
                            start=(c == 0), stop=(c == CHW - 1))
                    else:
                        nc.tensor.matmul(
                            out=pw[:, :1],
                            lhsT=sslab[:, rb0 + c * 112:rb0 + c * 112 + 128],
                            rhs=wslab[:, cc:cc + 1],
                            start=(c == 0), stop=(c == CHW - 1))
                if g == NG - 1:
                    state.pop(("sl", w))
                    if w % 2 == 1:
                        state.pop(("pair", w // 2))

            def tail1(i, w):
                """scatter-mean scale + transpose."""
                slim = i == 4
                nd = 1 if slim else 128
                pw = state.pop(("pw", w))
                argm = sb.tile([WN, nd], bf16,
                               tag="argm4" if slim else "argm", bufs=2)
                # drain + scatter-mean scale in one ACT op (per-partition
                # scale port carries 1/max(cnt,1))
                nc.scalar.activation(
                    argm[:], pw[:WN, :nd],
                    mybir.ActivationFunctionType.Identity,
                    scale=invc_sb[:, w:w + 1])
                pt = ppt.tile([nd, WN], bf16, tag="pt")
                nc.tensor.transpose(pt[:], argm[:], ident[:WN, :WN])
                state[("pt", w)] = pt

            def tail2(i, w):
                """aggregate -> node update -> new hT (or sigmoid out)."""
                slim = i == 4
                dout = 1 if slim else D
                nd = 1 if slim else 128
                pt = state.pop(("pt", w))
                aggt = sb.tile([128, WN], bf16, tag="aggt", bufs=2)
                nc.scalar.copy(aggt[:nd, :], pt[:])
                wc = slice(w * WN, (w + 1) * WN)
                pupd = pnode.tile([128, 128], f32, tag="pnode")
                nc.tensor.matmul(out=pupd[:dout, :WN], lhsT=wnt_sb[i][:],
                                 rhs=hT[:, wc], start=True, stop=False)
                nc.tensor.matmul(out=pupd[:dout, :WN], lhsT=wnb_sb[i][:],
                                 rhs=aggt[:], start=False, stop=True)
                if not slim:
                    nc.scalar.activation(
                        hT[:, wc], pupd[:, :WN],
                        mybir.ActivationFunctionType.Relu,
                        bias=bn_sb[i][:])
                else:
                    out_t = sb.tile([1, WN], f32, tag="out_t")
                    nc.scalar.activation(
                        out_t[:], pupd[:1, :WN],
                        mybir.ActivationFunctionType.Sigmoid,
                        bias=bn_sb[i][:])
                    nc.sync.dma_start(out_ext[:, wc], out_t[:])

            def tail3(i, w):
                """U/V of block i+1 for window w + segment AllGathers."""
                if i < 4:
                    uv_phase(i + 1, w)
                    if w == NWIN - 1:
                        ag_all(i + 1)

            def wee_bcast(i):
                """WeE of block i into rows 112:128 of every vw col block."""
                nc.sync.dma_start(
                    vw[WN:128, :].rearrange("p (w d) -> p w d", w=NWIN),
                    inp[f"wee{i}"][:, None, :].to_broadcast([ED, NWIN, 128]))

            # ---- prologue: UV of block 1 from x ----
            # five UV windows in flight: pnode + 2 pexp buffers x 2 slots
            wee_bcast(1)
            pe_hold = None
            for w in range(NWIN):
                m = w % 5
                if m == 0:
                    uv_phase(1, w)
                else:
                    if m in (1, 3):
                        pe_hold = pexp.tile([128, GRP * 128], f32, tag="pe",
                                            name="puv_alt")
                        uv_phase(1, w, puv=pe_hold[:, 0:256])
                    else:
                        uv_phase(1, w, puv=pe_hold[:, 256:512])
            ag_all(1)

            # ---- pipelined blocks ----
            for i in range(1, 5):
                for w in range(PF):
                    issue_loads(i, w)
                # flat stream of group-steps with skew:
                #   exp(G) | add(G-1) | relu(G-2) | red(G-3)
                # tails for window w ride at steps (w+1, 3), (w+1, 4),
                # (w+2, 0) of the stream.
                TOT = (NWIN + 2) * NG  # flush room
                for G in range(TOT):
                    w, g = divmod(G, NG)
                    if g == 0 and w + PF < NWIN:
                        issue_loads(i, w + PF)
                    if w < NWIN:
                        exp_group(i, w, g)
                    for (dk, fn) in ((1, add_group), (2, relu_group),
                                     (3, red_group)):
                        Gp = G - dk
                        if Gp >= 0:
                            wp, gp = divmod(Gp, NG)
                            if wp < NWIN:
                                fn(i, wp, gp)
                    # staggered tails: tail1(w-1)@g3, tail2(w-1)@g4,
                    # tail3(w-2)@g0
                    if g == 3 and 0 <= w - 1 < NWIN:
                        tail1(i, w - 1)
                    if g == 4 and 0 <= w - 1 < NWIN:
                        tail2(i, w - 1)
                    if g == 0 and 0 <= w - 2 < NWIN:
                        tail3(i, w - 2)
                if i < 3:
                    wee_bcast(i + 1)

    nc.finalize()
    return nc


_NC_CACHE = {}


def kernel(**inputs):
    from concourse.bass_utils import run_bass_kernel_spmd

    in_maps = _prep_inputs(inputs)
    if "nc" not in _NC_CACHE:
        _NC_CACHE["nc"] = _build()
    nc = _NC_CACHE["nc"]
    res = run_bass_kernel_spmd(nc, in_maps, core_ids=list(range(NCORES)))
    outs = [res.results[c]["out"].reshape(-1) for c in range(NCORES)]
    return np.concatenate(outs)[:N].reshape(N, 1).astype(np.float32)


# revision 61
# speedup vs baseline: 1.1429x; 1.0603x over previous
"""Distributed Trainium2 kernel for the 4-block GNN (nn_ActorGNN).

Strategy (edge-parallel, dst-sharded), v3:
  - Pad N=100000 -> NP=100352 = 8 * 12544 nodes; core c owns nodes
    [c*12544, (c+1)*12544).  Node features live transposed in SBUF (H^T,
    bf16).
  - Algebra: the edge MLP  relu([x_src|x_dst|ea] @ We + be)  is split as
    relu(U[src] + V[dst] + ea@WeE + be) with U = x@WeS, V = x@WeD computed
    per node shard (cheap N-side matmuls).
  - U is AllGathered (fp8) in two segments (second half overlaps the tail
    windows of the previous block); every core gathers arbitrary source
    rows with SWDGE indirect DMA; V/agg stay core-local (edges live on the
    core that owns their destination).
  - Edges are grouped by destination window of 112 nodes and padded to a
    uniform 30 chunks x 128 edges per window (SPMD-uniform).  112 was
    chosen so the expand matmul's contraction packs [S^T(112) ; ea(16)]
    against [V_win(112) ; WeE(16)] - the edge-attr matmul rides along for
    free.  Per chunk PE does two matmuls: the merged expand (fp8 one-hot
    lhsT x bf16 vw) and the one-hot segment-reduce; the gathered U rows
    are folded in by the vector engine while it drains the expand PSUM to
    SBUF (bf16), and the scalar engine applies relu -> fp8.
  - The per-window dataflow is software-pipelined as a flat stream of
    chunk-groups with skew  exp(G) | add(G-1) | relu(G-2) | reduce(G-3)
    so the PE never waits for the DVE/ACT round-trip, and the per-window
    tail (scatter-mean, node update, next-block U/V) is staggered across
    the following window's group steps.
"""

import numpy as np
import ml_dtypes

BF16 = ml_dtypes.bfloat16
F8 = ml_dtypes.float8_e4m3

N = 100_000
E = 3_200_000
D = 128
ED = 16
NCORES = 8
NS = 12_544           # nodes per core
NP = NS * NCORES      # padded node count
WN = 112              # nodes per window (112 + 16 ea rows = 128 = PE K)
NWIN = NS // WN       # 112 windows per core
CHW = 30              # chunks (of 128 edges) per window, uniform
GRP = 6               # chunks per relu/expand-psum group
NG = CHW // GRP       # groups per window (5)
ES = NWIN * CHW * 128  # padded edge slots per core
WQ = 4                # windows per srcg slab load
NSEG = 2              # AllGather segments
NS2 = NS // NSEG      # AllGather segment size (nodes)
WSEG = NWIN // NSEG   # windows per AG segment
PF = 4                # windows of prefetch for edge slabs


def _seg_remap(g):
    """Map global node id -> row in the segment-ordered u_full layout."""
    r, j = g // NS, g % NS
    s, j2 = j // NS2, j % NS2
    return s * (NP // NSEG) + r * NS2 + j2


# ---------------------------------------------------------------------------
# host-side preparation
# ---------------------------------------------------------------------------

def _prep_edges(edge_index, edge_attr):
    """Distribute edges to cores/windows; build per-core slot arrays."""
    src = edge_index[0].astype(np.int64)
    dst = edge_index[1].astype(np.int64)

    cnt = np.bincount(dst, minlength=NP).astype(np.float32)
    invc_full = 1.0 / np.maximum(cnt, 1.0)

    core = dst // NS
    win = (dst % NS) // WN
    l = dst % WN

    per_core = []
    for c in range(NCORES):
        m = core == c
        s_c, w_c, l_c = src[m], win[m], l[m]
        order = np.argsort(w_c, kind="stable")
        s_c, w_c, l_c = s_c[order], w_c[order], l_c[order]
        ea_c = edge_attr[m][order]

        counts = np.bincount(w_c, minlength=NWIN)
        assert counts.max() <= CHW * 128, f"window overflow: {counts.max()}"
        starts = np.concatenate([[0], np.cumsum(counts)])

        # slot arrays, (chunk, partition) order inside each window
        srcg = np.zeros((128, NWIN * CHW), dtype=np.int32)
        lcol = np.full((128, NWIN * CHW), -1, dtype=np.int64)
        eat = np.zeros((ED, ES), dtype=np.float32)

        for w in range(NWIN):
            k = counts[w]
            sl = slice(starts[w], starts[w + 1])
            # sort window edges by src: gather descriptors walk HBM in
            # ascending address order (better DRAM page locality)
            ow = np.argsort(s_c[sl], kind="stable")
            sw, lw, ew_ = s_c[sl][ow], l_c[sl][ow], ea_c[sl][ow]
            j = np.arange(k)
            ch = w * CHW + j // 128
            p = j % 128
            srcg[p, ch] = sw
            lcol[p, ch] = lw
            eat[:, ch * 128 + p] = ew_.T

        # stk: stacked stationary [S^T(112) ; ea(16)] per edge slot, fp8
        stk = np.zeros((128, ES), dtype=F8)
        lflat = lcol.T.reshape(-1)  # slot s = g*128 + p
        pos_valid = np.nonzero(lflat >= 0)[0]
        stk[lflat[pos_valid], pos_valid] = 1.0
        stk[WN:] = eat.astype(F8)

        # ssl: reduce one-hot S per chunk [slot(128) x l(112)], fp8, packed
        # at 112-col pitch.  The reduce LDWEIGHTS reads a 128-col view that
        # overlaps 16 cols into the next chunk (junk accumulates into pw
        # rows 112:128, which are never read) so FWL still triggers.
        ssl = np.zeros((128, NWIN * CHW * 112 + 16), dtype=F8)
        pp = pos_valid % 128
        cc = pos_valid // 128
        ssl[pp, cc * 112 + lflat[pos_valid]] = 1.0

        # retile window-PAIR-major so each pair's slab is one contiguous
        # region in HBM with 2x-sized per-partition descriptors
        stk = np.ascontiguousarray(
            stk.reshape(128, NWIN // 2, 2 * CHW * 128).transpose(1, 0, 2))
        sslw = np.zeros((NWIN // 2, 128, 2 * CHW * 112 + 16), dtype=F8)
        for p in range(NWIN // 2):
            sslw[p, :, :2 * CHW * 112] = \
                ssl[:, p * 2 * CHW * 112:(p + 1) * 2 * CHW * 112]
        ssl = sslw

        per_core.append(
            dict(
                srcg=srcg.astype(np.int32),
                stk=stk,
                ssl=ssl,
                invc=invc_full[c * NS:(c + 1) * NS].reshape(NWIN, WN).T.copy(),
            )
        )
    return per_core


def _prep_inputs(inputs):
    x = inputs["x"]
    xp = np.zeros((NP, D), dtype=np.float32)
    xp[:N] = x
    per_core_edges = _prep_edges(np.asarray(inputs["edge_index"]),
                                 np.asarray(inputs["edge_attr"]))

    ones1 = np.ones((1, 128), dtype=np.float32)

    blocks = []
    for i in range(1, 5):
        We = np.asarray(inputs[f"We{i}"], np.float32)
        be = np.asarray(inputs[f"be{i}"], np.float32)
        Wn = np.asarray(inputs[f"Wn{i}"], np.float32)
        bn = np.asarray(inputs[f"bn{i}"], np.float32)
        din = We.shape[0] - ED
        din //= 2
        dout = We.shape[1]
        WeS, WeD, WeE = We[:din], We[din:2 * din], We[2 * din:]
        # pad dout -> 128
        wesd = np.zeros((128, 256), np.float32)
        wesd[:din, :dout] = WeS
        wesd[:din, 128:128 + dout] = WeD
        berow = np.zeros((1, 256), np.float32)
        berow[0, 128:128 + dout] = be
        wee = np.zeros((ED, 128), np.float32)
        wee[:, :dout] = WeE
        wnt = np.zeros((128, dout), np.float32)
        wnt[:din] = Wn[:din]
        wnb = np.zeros((128, dout), np.float32)
        wnb[:dout] = Wn[din:]
        bncol = bn.reshape(dout, 1).astype(np.float32)
        b = dict(wesd=wesd.astype(BF16), berow=berow.astype(BF16),
                 wee=wee.astype(BF16), wnt=wnt.astype(BF16),
                 wnb=wnb.astype(BF16), bn=bncol)
        if i == 4:
            # slim block-4 params: dout=1; pad the U/V pair to 64 columns so
            # the bounce rows are 128B (sub-burst HBM writes trigger RMW
            # stalls that throttled every DMA engine during block 3)
            wesd4b = np.zeros((128, 64), np.float32)
            wesd4b[:din, 0] = WeS[:, 0]
            wesd4b[:din, 1] = WeD[:, 0]
            berow4b = np.zeros((1, 64), np.float32)
            berow4b[0, 1] = be[0]
            b["wesdb"] = wesd4b.astype(BF16)
            b["berowb"] = berow4b.astype(BF16)
            b["weeb"] = np.tile(WeE[:, :1], (1, NWIN)).astype(BF16)
        blocks.append(b)

    # block-1 U/V depend only on inputs -> precompute on host, skipping the
    # on-device prologue UV sweep and the first AllGather entirely
    xb = xp.astype(BF16).astype(np.float32)
    We1 = np.asarray(inputs["We1"], np.float32)
    be1 = np.asarray(inputs["be1"], np.float32)
    WeS1, WeD1, WeE1 = We1[:D], We1[D:2 * D], We1[2 * D:]
    u1_full = (xb @ WeS1).astype(F8)                       # [NP, 128]
    v1_full = (xb @ WeD1 + be1).astype(BF16)               # [NP, 128]

    in_maps = []
    for c in range(NCORES):
        v1c = v1_full[c * NS:(c + 1) * NS].astype(np.float32)
        vw1 = np.zeros((128, NWIN * 128), np.float32)
        vw1[:WN] = v1c.reshape(NWIN, WN, 128).transpose(1, 0, 2).reshape(
            WN, NWIN * 128)
        vw1[WN:] = np.tile(WeE1, (1, NWIN))
        m = dict(
            xT=xp[c * NS:(c + 1) * NS].T.astype(BF16).copy(),
            srcg=per_core_edges[c]["srcg"],
            stk=per_core_edges[c]["stk"],
            ssl=per_core_edges[c]["ssl"],
            invc=per_core_edges[c]["invc"],
            ones1=ones1.astype(BF16),
            u1=u1_full,
            vw1=vw1.astype(BF16),
        )
        for i, b in enumerate(blocks, 1):
            for k, v in b.items():
                m[f"{k}{i}"] = v
        in_maps.append(m)
    return in_maps


# ---------------------------------------------------------------------------
# bass program
# ---------------------------------------------------------------------------

def _build():
    from concourse import bacc, bass, mybir, tile
    from concourse.masks import make_identity

    f32 = mybir.dt.float32
    bf16 = mybir.dt.bfloat16
    fp8 = mybir.dt.float8e4
    i32 = mybir.dt.int32

    nc = bacc.Bacc("TRN2", num_devices=NCORES)

    inp = {}
    for name, shape, dt in [
        ("xT", [128, NS], bf16),
        ("srcg", [128, NWIN * CHW], i32),
        ("stk", [NWIN // 2, 128, 2 * CHW * 128], fp8),
        ("ssl", [NWIN // 2, 128, 2 * CHW * 112 + 16], fp8),
        ("invc", [WN, NWIN], f32),
        ("ones1", [1, 128], bf16),
        ("u1", [NP, 128], fp8),
        ("vw1", [128, NWIN * 128], bf16),
    ]:
        inp[name] = nc.dram_tensor(name, shape, dt, kind="ExternalInput")
    for i in range(1, 5):
        dout = 1 if i == 4 else D
        for name, shape, dt in [
            (f"wesd{i}", [128, 256], bf16),
            (f"berow{i}", [1, 256], bf16),
            (f"wee{i}", [ED, 128], bf16),
            (f"wnt{i}", [128, dout], bf16),
            (f"wnb{i}", [128, dout], bf16),
            (f"bn{i}", [dout, 1], f32),
        ]:
            inp[name] = nc.dram_tensor(name, shape, dt, kind="ExternalInput")
    for name, shape, dt in [
        ("wesdb4", [128, 64], bf16),
        ("berowb4", [1, 64], bf16),
        ("weeb4", [ED, NWIN], bf16),
    ]:
        inp[name] = nc.dram_tensor(name, shape, dt, kind="ExternalInput")
    out_ext = nc.dram_tensor("out", [1, NS], f32, kind="ExternalOutput")

    with tile.TileContext(nc) as tc:
        with tc.tile_pool(name="res", bufs=1) as res, \
             tc.tile_pool(name="sb", bufs=2) as sb, \
             tc.tile_pool(name="pexp", bufs=2, space="PSUM") as pexp, \
             tc.tile_pool(name="pnode", bufs=1, space="PSUM") as pnode, \
             tc.tile_pool(name="ppw", bufs=2, space="PSUM") as ppw, \
             tc.tile_pool(name="ppt", bufs=1, space="PSUM") as ppt, \
             tc.tile_pool(name="dram", bufs=1, space="DRAM") as dram:

            # resident tensors
            hT = res.tile([128, NS], bf16)         # node features, transposed
            # [V_win(112) ; WeE(16)] per window, window w at cols w*128
            vw = res.tile([128, NWIN * 128], bf16)
            vw4 = res.tile([128, NWIN], bf16)
            invc_sb = res.tile([WN, NWIN], f32)
            ones_sb = res.tile([1, 128], bf16)
            ident = res.tile([128, 128], bf16)

            nc.sync.dma_start(hT[:], inp["xT"][:])
            nc.sync.dma_start(invc_sb[:], inp["invc"][:])
            nc.sync.dma_start(ones_sb[:], inp["ones1"][:])
            nc.sync.dma_start(vw4[WN:128, :], inp["weeb4"][:])
            make_identity(nc, ident[:])

            # per-block weights, all resident
            wesd_sb, berow_sb, wee_sb, wnt_sb, wnb_sb, bn_sb = \
                {}, {}, {}, {}, {}, {}
            for i in range(1, 5):
                dout = 1 if i == 4 else D
                if i < 4:
                    wesd_sb[i] = res.tile([128, 256], bf16, tag=f"wesd{i}", name=f"wesd{i}")
                    berow_sb[i] = res.tile([1, 256], bf16, tag=f"berow{i}", name=f"berow{i}")
                    wee_sb[i] = res.tile([ED, 128], bf16, tag=f"wee{i}", name=f"wee{i}")
                    nc.sync.dma_start(wesd_sb[i][:], inp[f"wesd{i}"][:])
                    nc.sync.dma_start(berow_sb[i][:], inp[f"berow{i}"][:])
                    nc.sync.dma_start(wee_sb[i][:], inp[f"wee{i}"][:])
                else:
                    wesd_sb[i] = res.tile([128, 64], bf16, tag="wesd4", name="wesd4")
                    berow_sb[i] = res.tile([1, 64], bf16, tag="berow4", name="berow4")
                    nc.sync.dma_start(wesd_sb[i][:], inp["wesdb4"][:])
                    nc.sync.dma_start(berow_sb[i][:], inp["berowb4"][:])
                wnt_sb[i] = res.tile([128, dout], bf16, tag=f"wnt{i}", name=f"wnt{i}")
                wnb_sb[i] = res.tile([128, dout], bf16, tag=f"wnb{i}", name=f"wnb{i}")
                bn_sb[i] = res.tile([dout, 1], f32, tag=f"bn{i}", name=f"bn{i}")
                nc.sync.dma_start(wnt_sb[i][:], inp[f"wnt{i}"][:])
                nc.sync.dma_start(wnb_sb[i][:], inp[f"wnb{i}"][:])
                nc.sync.dma_start(bn_sb[i][:], inp[f"bn{i}"][:])

            u_bounce = dram.tile([NS, 128], fp8)
            u4_bounce = dram.tile([NS, 64], bf16, name="u4_bounce",
                                  tag="u4_bounce")
            u4_pack = dram.tile([NS, 2], bf16, name="u4_pack",
                                tag="u4_pack")

            u_fulls = {1: inp["u1"]}
            for i in (3, 2):
                uf = dram.tile([NP, 128], fp8, addr_space="Shared",
                               name=f"u_full{i}", tag=f"u_full{i}")
                u_fulls[i] = uf
            u4f = dram.tile([NP, 2], bf16, addr_space="Shared",
                            name="u4_full", tag="u4_full")
            u_fulls[4] = u4f

            def uv_phase(i, w, puv=None):
                """Compute U/V of block i for window w from current hT."""
                slim = i == 4
                wc = slice(w * WN, (w + 1) * WN)
                if puv is None:
                    puv = pnode.tile([128, 256], f32, tag="pnode",
                                     name="puv")
                nuv = 64 if slim else 256
                nc.tensor.matmul(out=puv[:WN, :nuv],
                                 lhsT=hT[:, wc], rhs=wesd_sb[i][:],
                                 start=True, stop=False)
                nc.tensor.matmul(out=puv[:WN, :nuv],
                                 lhsT=ones_sb[:, :WN],
                                 rhs=berow_sb[i][:], start=False, stop=True)
                if not slim:
                    utile = sb.tile([WN, 128], fp8, tag="utile", bufs=6)
                    nc.scalar.copy(utile[:], puv[:WN, :128])
                    nc.vector.tensor_copy(vw[:WN, w * 128:(w + 1) * 128],
                                          puv[:WN, 128:256])
                    nc.sync.dma_start(u_bounce[wc, :], utile[:])
                else:
                    utile = sb.tile([WN, 64], bf16, tag="utile4", bufs=6)
                    nc.scalar.copy(utile[:], puv[:WN, :64])
                    nc.vector.tensor_copy(vw4[:WN, w:w + 1], puv[:WN, 1:2])
                    nc.sync.dma_start(u4_bounce[wc, :], utile[:])

            def ag_all(i):
                """AllGather block i's U into u_fulls[i]."""
                full = u_fulls[i]
                if i == 4:
                    # repack the 128B-row bounce into the compact layout the
                    # AllGather/gather expects.  Round-trip through SBUF so
                    # the DRAM write side is 392B-contiguous per partition
                    # (a direct strided copy emits 12544 4-byte writes that
                    # each trigger an HBM read-modify-write)
                    u4sb = sb.tile([128, (NS // 128) * 2], bf16,
                                   tag="u4sb", name="u4sb")
                    nc.sync.dma_start(
                        u4sb[:].rearrange("p (k t) -> p k t", t=2),
                        u4_bounce[:, 0:2].rearrange("(p k) t -> p k t",
                                                    p=128))
                    nc.sync.dma_start(
                        u4_pack[:].rearrange("(p k) t -> p k t", p=128),
                        u4sb[:].rearrange("p (k t) -> p k t", t=2))
                    in_ap = u4_pack[:]
                else:
                    in_ap = u_bounce[:]
                nc.gpsimd.collective_compute(
                    "AllGather", mybir.AluOpType.bypass,
                    replica_groups=[list(range(NCORES))],
                    ins=[in_ap.opt()],
                    outs=[full.opt()],
                )

            # ------------------------------------------------------------
            # pipelined block body
            # ------------------------------------------------------------
            state = {}

            def issue_loads(i, w):
                """Prefetch edge slabs for window w of block i."""
                slim = i == 4
                if w % WQ == 0:
                    srcg_sl = sb.tile([128, WQ * CHW], i32, tag="srcg",
                                      bufs=3)
                    nc.sync.dma_start(
                        srcg_sl[:],
                        inp["srcg"][:, w * CHW:(w + WQ) * CHW])
                    state["srcg"] = srcg_sl
                w0 = (w % WQ) * CHW
                uslab = sb.tile(
                    [128, CHW * 2] if slim else [128, CHW * 128],
                    bf16 if slim else fp8,
                    tag="uslab4" if slim else "uslab", bufs=PF + 2)
                nc.gpsimd.indirect_dma_start(
                    out=uslab[:],
                    out_offset=None,
                    in_=u_fulls[i][:],
                    in_offset=bass.IndirectOffsetOnAxis(
                        ap=state["srcg"][:, w0:w0 + CHW], axis=0),
                )
                if w % 2 == 0:
                    stslab = sb.tile([128, 2 * CHW * 128], fp8, tag="stslab",
                                     bufs=PF // 2 + 1)
                    nc.sync.dma_start(stslab[:], inp["stk"][w // 2])
                    sslab = sb.tile([128, 2 * CHW * 112 + 16], fp8,
                                    tag="sslab", bufs=PF // 2 + 2)
                    nc.sync.dma_start(sslab[:], inp["ssl"][w // 2])
                    state[("pair", w // 2)] = (stslab, sslab)
                state[("sl", w)] = (uslab,) + state[("pair", w // 2)]

            def exp_group(i, w, g):
                slim = i == 4
                _, stslab, _ = state[("sl", w)]
                g0 = g * GRP
                sb0 = (w % 2) * CHW * 128
                if not slim:
                    pe_ = pexp.tile([128, GRP * 128], f32, tag="pe")
                    for c in range(g0, g0 + GRP):
                        r = (c - g0) * 128
                        nc.tensor.matmul(
                            out=pe_[:, r:r + 128],
                            lhsT=stslab[:, sb0 + c * 128:sb0 + (c + 1) * 128],
                            rhs=vw[:, w * 128:(w + 1) * 128],
                            start=True, stop=True)
                else:
                    pe_ = pexp.tile([128, GRP], f32, tag="pe")
                    for c in range(g0, g0 + GRP):
                        nc.tensor.matmul(
                            out=pe_[:, c - g0:c - g0 + 1],
                            lhsT=stslab[:, sb0 + c * 128:sb0 + (c + 1) * 128],
                            rhs=vw4[:, w:w + 1],
                            start=True, stop=True)
                state[("pe", w, g)] = pe_

            def add_group(i, w, g):
                """Drain expand PSUM + add gathered U rows -> SBUF bf16."""
                slim = i == 4
                uslab, _, _ = state[("sl", w)]
                pe_ = state.pop(("pe", w, g))
                g0 = g * GRP
                if not slim:
                    smt = sb.tile([128, GRP * 128], bf16, tag="smt", bufs=4)
                    nc.vector.tensor_tensor(
                        out=smt[:], in0=pe_[:],
                        in1=uslab[:, g0 * 128:(g0 + GRP) * 128],
                        op=mybir.AluOpType.add)
                else:
                    smt = sb.tile([128, GRP], bf16, tag="smt4", bufs=4)
                    u4r = uslab[:].rearrange("p (c t) -> p c t", t=2)
                    nc.vector.tensor_tensor(
                        out=smt[:], in0=pe_[:],
                        in1=u4r[:, g0:g0 + GRP, 0:1],
                        op=mybir.AluOpType.add)
                state[("sm", w, g)] = smt

            def relu_group(i, w, g):
                slim = i == 4
                smt = state.pop(("sm", w, g))
                if not slim:
                    wslab = sb.tile([128, GRP * 128], bf16, tag="wslab",
                                    bufs=4)
                else:
                    wslab = sb.tile([128, GRP], bf16, tag="wslab4", bufs=4)
                if g == NG - 1 and not slim:
                    # one group per window on DVE (4x mode, 16-bit SBUF)
                    nc.vector.tensor_scalar(
                        out=wslab[:], in0=smt[:], scalar1=0.0, scalar2=None,
                        op0=mybir.AluOpType.max)
                else:
                    nc.scalar.activation(
                        wslab[:], smt[:], mybir.ActivationFunctionType.Relu)
                state[("ws", w, g)] = wslab

            def red_group(i, w, g):
                slim = i == 4
                _, _, sslab = state[("sl", w)]
                wslab = state.pop(("ws", w, g))
                if g == 0:
                    state[("pw", w)] = ppw.tile([128, 128], f32, tag="pw",
                                                name="pw")
                pw = state[("pw", w)]
                g0 = g * GRP
                rb0 = (w % 2) * CHW * 112
                for c in range(g0, g0 + GRP):
                    cc = c - g0
                    if not slim:
                        nc.tensor.matmul(
                            out=pw[:, :],
                            lhsT=sslab[:, rb0 + c * 112:rb0 + c * 112 + 128],
                            rhs=wslab[:, cc * 128:(cc + 1) * 128],
                            start=(c == 0), stop=(c == CHW - 1))
                    else:
                        nc.tensor.matmul(
                            out=pw[:, :1],
                            lhsT=sslab[:, rb0 + c * 112:rb0 + c * 112 + 128],
                            rhs=wslab[:, cc:cc + 1],
                            start=(c == 0), stop=(c == CHW - 1))
                if g == NG - 1:
                    state.pop(("sl", w))
                    if w % 2 == 1:
                        state.pop(("pair", w // 2))

            def tail1(i, w):
                """scatter-mean scale + transpose."""
                slim = i == 4
                nd = 1 if slim else 128
                pw = state.pop(("pw", w))
                argm = sb.tile([WN, nd], bf16,
                               tag="argm4" if slim else "argm", bufs=2)
                # drain + scatter-mean scale in one ACT op (per-partition
                # scale port carries 1/max(cnt,1))
                nc.scalar.activation(
                    argm[:], pw[:WN, :nd],
                    mybir.ActivationFunctionType.Identity,
                    scale=invc_sb[:, w:w + 1])
                pt = ppt.tile([nd, WN], bf16, tag="pt")
                nc.tensor.transpose(pt[:], argm[:], ident[:WN, :WN])
                state[("pt", w)] = pt

            def tail2(i, w):
                """aggregate -> node update -> new hT (or sigmoid out)."""
                slim = i == 4
                dout = 1 if slim else D
                nd = 1 if slim else 128
                pt = state.pop(("pt", w))
                aggt = sb.tile([128, WN], bf16, tag="aggt", bufs=2)
                nc.scalar.copy(aggt[:nd, :], pt[:])
                wc = slice(w * WN, (w + 1) * WN)
                pupd = pnode.tile([128, 128], f32, tag="pnode")
                nc.tensor.matmul(out=pupd[:dout, :WN], lhsT=wnt_sb[i][:],
                                 rhs=hT[:, wc], start=True, stop=False)
                nc.tensor.matmul(out=pupd[:dout, :WN], lhsT=wnb_sb[i][:],
                                 rhs=aggt[:], start=False, stop=True)
                if not slim:
                    nc.scalar.activation(
                        hT[:, wc], pupd[:, :WN],
                        mybir.ActivationFunctionType.Relu,
                        bias=bn_sb[i][:])
                else:
                    out_t = sb.tile([1, WN], f32, tag="out_t")
                    nc.scalar.activation(
                        out_t[:], pupd[:1, :WN],
                        mybir.ActivationFunctionType.Sigmoid,
                        bias=bn_sb[i][:])
                    nc.sync.dma_start(out_ext[:, wc], out_t[:])

            def tail3(i, w):
                """U/V of block i+1 for window w + segment AllGathers."""
                if i < 4:
                    uv_phase(i + 1, w)
                    if w == NWIN - 1:
                        ag_all(i + 1)

            def wee_bcast(i):
                """WeE of block i into rows 112:128 of every vw col block."""
                nc.sync.dma_start(
                    vw[WN:128, :].rearrange("p (w d) -> p w d", w=NWIN),
                    inp[f"wee{i}"][:, None, :].to_broadcast([ED, NWIN, 128]))

            # ---- prologue: block-1 U/V were precomputed on the host ----
            nc.sync.dma_start(vw[:], inp["vw1"][:])

            # ---- pipelined blocks ----
            for i in range(1, 5):
                for w in range(PF):
                    issue_loads(i, w)
                # flat stream of group-steps with skew:
                #   exp(G) | add(G-1) | relu(G-2) | red(G-3)
                # tails for window w ride at steps (w+1, 3), (w+1, 4),
                # (w+2, 0) of the stream.
                TOT = (NWIN + 2) * NG  # flush room
                for G in range(TOT):
                    w, g = divmod(G, NG)
                    if g == 0 and w + PF < NWIN:
                        issue_loads(i, w + PF)
                    if w < NWIN:
                        exp_group(i, w, g)
                    for (dk, fn) in ((1, add_group), (2, relu_group),
                                     (3, red_group)):
                        Gp = G - dk
                        if Gp >= 0:
                            wp, gp = divmod(Gp, NG)
                            if wp < NWIN:
                                fn(i, wp, gp)
                    # staggered tails: tail1(w-1)@g3, tail2(w-1)@g4,
                    # tail3(w-2)@g0
                    if g == 3 and 0 <= w - 1 < NWIN:
                        tail1(i, w - 1)
                    if g == 4 and 0 <= w - 1 < NWIN:
                        tail2(i, w - 1)
                    if g == 0 and 0 <= w - 2 < NWIN:
                        tail3(i, w - 2)
                if i < 3:
                    wee_bcast(i + 1)

    nc.finalize()
    return nc


_NC_CACHE = {}


def kernel(**inputs):
    from concourse.bass_utils import run_bass_kernel_spmd

    in_maps = _prep_inputs(inputs)
    if "nc" not in _NC_CACHE:
        _NC_CACHE["nc"] = _build()
    nc = _NC_CACHE["nc"]
    res = run_bass_kernel_spmd(nc, in_maps, core_ids=list(range(NCORES)))
    outs = [res.results[c]["out"].reshape(-1) for c in range(NCORES)]
    return np.concatenate(outs)[:N].reshape(N, 1).astype(np.float32)
